# revision 1
# baseline (speedup 1.0000x reference)
"""GraphSAGE 3-layer GNN forward pass on 8 Trainium2 NeuronCores.

Sharding: nodes split by range across 8 cores (graph/data parallel).
Per layer the message table z = h @ Wl is computed shard-wise (node-major
rows) and AllGathered into a replicated DRAM table; each core aggregates
the edges whose dst is in its shard: dma_gather pulls z[src] rows (256B)
into SBUF and a one-hot matmul on the tensor engine does the segment-sum
into PSUM (feature-major for layers 1/2, node-major for layer 3).
Mean-normalization (1/deg), the self term h @ Wr, BatchNorm (stats
AllReduced), ReLU and log_softmax run on vector/scalar engines.
int16 gather indices only reach 32768 rows, so the table is processed in
4 buckets of 25088 rows with (dst-tile, bucket)-pure edge chunks.
"""

import math
import numpy as np

# ---------------- problem constants (hardcoded per contract) ----------------
N = 100000
E = 1600000
FIN = 200
NCORES = 8
NPC = N // NCORES            # 12500 nodes per core
NT = 98                      # dst tiles of 128 nodes per core
NPAD = NT * 128              # 12544
SHARD = NPAD                 # table rows contributed per core
TROWS = SHARD * NCORES       # 100352
NBUCK = 4
BROWS = TROWS // NBUCK       # 25088 (< 32768, int16-safe)
F1, F2, F3 = 64, 32, 17
EPS = 1e-5

# ---------------- tunables ----------------
NQ = 1                # SWDGE queues used for gather calls
NSQ = 4               # queues declared (allows NQ up to 4)
GSIZE = 8             # dst tiles per PSUM accumulation group
PBATCH = 8            # chunks per one-hot build DVE op


def _wrap16(idx_flat):
    """dma_gather index layout: position i -> partition i%16, col i//16,
    replicated across the 8 q7 core pairs (128 partitions)."""
    n = idx_flat.shape[0]
    w = idx_flat.reshape(n // 16, 16).T.copy()
    return np.tile(w, (8, 1))


def _preprocess(edge_index):
    src = np.asarray(edge_index[0], dtype=np.int64)
    dst = np.asarray(edge_index[1], dtype=np.int64)
    trow = (src // NPC) * SHARD + (src % NPC)   # global table row of src
    bucket = trow // BROWS
    rel = trow - bucket * BROWS

    dst_core = dst // NPC
    dloc = dst - dst_core * NPC
    tile_e = dloc >> 7
    dstrel_e = dloc & 127

    per_core = []
    needed = np.zeros((NCORES, NT, NBUCK), np.int64)
    for c in range(NCORES):
        m = dst_core == c
        key = tile_e[m] * NBUCK + bucket[m]
        order = np.argsort(key, kind="stable")
        cnts = np.bincount(key, minlength=NT * NBUCK).reshape(NT, NBUCK)
        per_core.append({
            "key": key[order],
            "rel": rel[m][order],
            "dstrel": dstrel_e[m][order],
            "cnt": np.bincount(dloc[m], minlength=NPC),
            "cnts": cnts,
        })
        needed[c] = (cnts + 127) >> 7
    csched = np.maximum(needed.max(axis=0), 1)   # [NT, NBUCK]

    groups = [list(range(g, min(g + GSIZE, NT))) for g in range(0, NT, GSIZE)]
    chunk_start = np.zeros((NT, NBUCK), np.int64)
    calls = []  # (bucket, chunk_qstart, nchunks, group_index)
    q = 0
    for gi, g in enumerate(groups):
        for b in range(NBUCK):
            nch = 0
            for t in g:
                chunk_start[t, b] = q + nch
                nch += int(csched[t, b])
            calls.append((b, q, nch, gi))
            q += nch
    nchunk = q
    tile_of_chunk = np.zeros(nchunk, np.int64)
    for t in range(NT):
        for b in range(NBUCK):
            s = chunk_start[t, b]
            tile_of_chunk[s:s + csched[t, b]] = t

    idx_all = np.zeros((NCORES, 128, nchunk * 8), np.int16)
    dstrel_all = np.full((NCORES, 128, nchunk), -1.0, np.float32)
    rcnt_nm = np.zeros((NCORES, 128, NT), np.float32)
    rcnt_row = np.zeros((NCORES, NPAD), np.float32)
    for c in range(NCORES):
        ck = per_core[c]
        seg_off = np.zeros(NT * NBUCK + 1, np.int64)
        seg_off[1:] = np.cumsum(ck["cnts"].reshape(-1))
        pos = np.arange(len(ck["key"])) - seg_off[ck["key"]]
        t_e = ck["key"] // NBUCK
        b_e = ck["key"] % NBUCK
        qg = chunk_start[t_e, b_e] + (pos >> 7)
        p = pos & 127
        idx_flat = np.zeros(nchunk * 128, np.int16)
        idx_flat[qg * 128 + p] = ck["rel"].astype(np.int16)
        idx_all[c] = _wrap16(idx_flat)
        dstrel_all[c][p, qg] = ck["dstrel"].astype(np.float32)
        rc_pad = np.ones(NPAD, np.float32)
        rc_pad[:NPC] = 1.0 / np.maximum(ck["cnt"], 1).astype(np.float32)
        rcnt_nm[c] = rc_pad.reshape(NT, 128).T
        rcnt_row[c] = rc_pad

    return {
        "csched": csched, "groups": groups, "calls": calls, "nchunk": nchunk,
        "chunk_start": chunk_start, "tile_of_chunk": tile_of_chunk,
        "idx_all": idx_all, "dstrel_all": dstrel_all,
        "rcnt_nm": rcnt_nm, "rcnt_row": rcnt_row,
    }


def _build_program(pp):
    import concourse.bacc as bacc
    import concourse.tile as tile
    import concourse.bass as bass
    import concourse.mybir as mybir

    f32 = mybir.dt.float32
    AX = mybir.AxisListType
    ALU = mybir.AluOpType
    ACT = mybir.ActivationFunctionType

    groups = pp["groups"]
    calls = pp["calls"]
    csched = pp["csched"]
    chunk_start = pp["chunk_start"]
    tile_of_chunk = pp["tile_of_chunk"]
    nchunk = pp["nchunk"]
    max_call_chunks = max(nc_ for (_, _, nc_, _) in calls)

    nc = bacc.Bacc("TRN2", target_bir_lowering=False, debug=False,
                   num_devices=NCORES, num_swdge_queues=NSQ)

    # ---------------- I/O ----------------
    t_xT = nc.dram_tensor("xT", [FIN, NPAD], f32, kind="ExternalInput")
    t_idx = nc.dram_tensor("gidx", [128, nchunk * 8], mybir.dt.int16, kind="ExternalInput")
    t_dstrel = nc.dram_tensor("dstrel", [128, nchunk], f32, kind="ExternalInput")
    t_rcnt_nm = nc.dram_tensor("rcnt_nm", [128, NT], f32, kind="ExternalInput")
    t_rcnt_fm = nc.dram_tensor("rcnt_fm", [64, NPAD], f32, kind="ExternalInput")
    t_iota = nc.dram_tensor("iota", [128, 128], f32, kind="ExternalInput")
    t_W1l = nc.dram_tensor("W1l", [FIN, F1], f32, kind="ExternalInput")
    t_W1r = nc.dram_tensor("W1r", [FIN, F1], f32, kind="ExternalInput")
    t_W2l = nc.dram_tensor("W2lp", [F1, 64], f32, kind="ExternalInput")
    t_W2r = nc.dram_tensor("W2r", [F1, F2], f32, kind="ExternalInput")
    t_W3l = nc.dram_tensor("W3lp", [F2, 64], f32, kind="ExternalInput")
    t_W3r = nc.dram_tensor("W3r", [F2, F3], f32, kind="ExternalInput")
    t_g1 = nc.dram_tensor("g1", [F1, 1], f32, kind="ExternalInput")
    t_be1 = nc.dram_tensor("be1", [F1, 1], f32, kind="ExternalInput")
    t_g2 = nc.dram_tensor("g2", [F2, 1], f32, kind="ExternalInput")
    t_be2 = nc.dram_tensor("be2", [F2, 1], f32, kind="ExternalInput")
    t_b3 = nc.dram_tensor("b3rep", [128, F3], f32, kind="ExternalInput")
    t_out = nc.dram_tensor("out", [NPAD, F3], f32, kind="ExternalOutput")

    shard1 = nc.dram_tensor("shard1", [SHARD, 64], f32, kind="Internal")
    shard2 = nc.dram_tensor("shard2", [SHARD, 64], f32, kind="Internal")
    shard3 = nc.dram_tensor("shard3", [SHARD, 64], f32, kind="Internal")
    zfull1 = nc.dram_tensor("zfull1", [TROWS, 64], f32, kind="Internal", addr_space="Shared")
    zfull2 = nc.dram_tensor("zfull2", [TROWS, 64], f32, kind="Internal", addr_space="Shared")
    zfull3 = nc.dram_tensor("zfull3", [TROWS, 64], f32, kind="Internal", addr_space="Shared")
    zrT1_d = nc.dram_tensor("zrT1", [64, NPAD], f32, kind="Internal")
    zrT2_d = nc.dram_tensor("zrT2", [F2, NPAD], f32, kind="Internal")
    zr3_d = nc.dram_tensor("zr3", [NPAD, F3], f32, kind="Internal")
    hT1_d = nc.dram_tensor("hT1", [64, NPAD], f32, kind="Internal")
    hT2_d = nc.dram_tensor("hT2", [F2, NPAD], f32, kind="Internal")
    bn_in1 = nc.dram_tensor("bn_in1", [F1, 2], f32, kind="Internal")
    bn_out1 = nc.dram_tensor("bn_out1", [F1, 2], f32, kind="Internal", addr_space="Shared")
    bn_in2 = nc.dram_tensor("bn_in2", [F2, 2], f32, kind="Internal")
    bn_out2 = nc.dram_tensor("bn_out2", [F2, 2], f32, kind="Internal", addr_space="Shared")

    RG = [list(range(NCORES))]
    GW = GSIZE * 128

    with tile.TileContext(nc) as tc:
        with tc.tile_pool(name="const", bufs=1) as constp, \
             tc.tile_pool(name="wpool", bufs=1) as wpool, \
             tc.tile_pool(name="stage", bufs=2) as stagep, \
             tc.tile_pool(name="sm3", bufs=3) as sm3p, \
             tc.tile_pool(name="slab", bufs=2) as slabp, \
             tc.tile_pool(name="gbuf", bufs=4) as gbufp, \
             tc.tile_pool(name="pbuf", bufs=3) as pbufp, \
             tc.tile_pool(name="zpsum", bufs=2, space="PSUM") as zpsum, \
             tc.tile_pool(name="spsum", bufs=2, space="PSUM") as spsum, \
             tc.tile_pool(name="small", bufs=1) as smallp:

            # ---- constants
            iota = constp.tile([128, 128], f32)
            nc.sync.dma_start(iota[:], t_iota.ap())
            idx_t = constp.tile([128, nchunk * 8], mybir.dt.int16)
            nc.sync.dma_start(idx_t[:], t_idx.ap())
            dstrel_t = constp.tile([128, nchunk], f32)
            nc.sync.dma_start(dstrel_t[:], t_dstrel.ap())
            rcnt_nm_t = constp.tile([128, NT], f32)
            nc.sync.dma_start(rcnt_nm_t[:], t_rcnt_nm.ap())
            b3rep = constp.tile([128, F3], f32)
            nc.sync.dma_start(b3rep[:], t_b3.ap())

            def wload(name, tt, shape):
                w = wpool.tile(shape, f32, tag=name)
                nc.sync.dma_start(w[:], tt)
                return w

            W1l_a = wload("w1la", t_W1l.ap()[:128], [128, F1])
            W1l_b = wload("w1lb", t_W1l.ap()[128:], [72, F1])
            W1r_a = wload("w1ra", t_W1r.ap()[:128], [128, F1])
            W1r_b = wload("w1rb", t_W1r.ap()[128:], [72, F1])
            W2l_t = wload("w2l", t_W2l.ap(), [F1, 64])
            W2r_t = wload("w2r", t_W2r.ap(), [F1, F2])
            W3l_t = wload("w3l", t_W3l.ap(), [F2, 64])
            W3r_t = wload("w3r", t_W3r.ap(), [F2, F3])
            g1_t = wload("g1", t_g1.ap(), [F1, 1])
            be1_t = wload("be1", t_be1.ap(), [F1, 1])
            g2_t = wload("g2", t_g2.ap(), [F2, 1])
            be2_t = wload("be2", t_be2.ap(), [F2, 1])

            # ================= layer-1 z phase =================
            with nc.named_scope("L1z"):
                for gi, g in enumerate(groups):
                    gw = len(g) * 128
                    c0 = g[0] * 128
                    xa = slabp.tile([128, GW], f32, tag="xa")
                    xb = slabp.tile([72, GW], f32, tag="xb")
                    nc.sync.dma_start(xa[:, :gw], t_xT.ap()[:128, c0:c0 + gw])
                    nc.sync.dma_start(xb[:, :gw], t_xT.ap()[128:, c0:c0 + gw])
                    zr_sl = stagep.tile([64, GW], f32, tag="zrslab")
                    for ti, t in enumerate(g):
                        xs_a = xa[:, ti * 128:(ti + 1) * 128]
                        xs_b = xb[:, ti * 128:(ti + 1) * 128]
                        pz = zpsum.tile([128, 128], f32, tag="zps")
                        nc.tensor.matmul(pz[:, :F1], xs_a, W1l_a[:], start=True, stop=False)
                        nc.tensor.matmul(pz[:, :F1], xs_b, W1l_b[:], start=False, stop=True)
                        zs = sm3p.tile([128, 64], f32, tag="zstage")
                        nc.scalar.copy(zs[:], pz[:, :F1])
                        nc.sync.dma_start(shard1.ap()[t * 128:(t + 1) * 128], zs[:])
                        pr = zpsum.tile([128, 128], f32, tag="zps")
                        nc.tensor.matmul(pr[:F1, :], W1r_a[:], xs_a, start=True, stop=False)
                        nc.tensor.matmul(pr[:F1, :], W1r_b[:], xs_b, start=False, stop=True)
                        nc.scalar.copy(zr_sl[:, ti * 128:(ti + 1) * 128], pr[:F1, :])
                    nc.sync.dma_start(zrT1_d.ap()[:, c0:c0 + gw], zr_sl[:, :gw])

            with nc.named_scope("AG1"):
                nc.gpsimd.collective_compute(
                    "AllGather", ALU.bypass, replica_groups=RG,
                    ins=[shard1.ap()], outs=[zfull1.ap()])

            # ========== generic gather/aggregate ==========
            def agg_layer(zfull, Fw, fm, zr_src, h_sink, scope, final_cb=None):
                stat_parts = []
                with nc.named_scope(scope):
                    cur_ps = None
                    for ci, (b, qs, nch, gi) in enumerate(calls):
                        g = groups[gi]
                        gw = len(g) * 128
                        c0 = g[0] * 128
                        if b == 0:
                            if fm:
                                cur_ps = spsum.tile([Fw, GW], f32, tag="sacc")
                            else:
                                cur_ps = spsum.tile([128, GSIZE * F3], f32, tag="sacc3")
                        ps = cur_ps
                        gb = gbufp.tile([128, max_call_chunks, 64], f32, tag="gb")
                        nc.gpsimd.dma_gather(
                            out_ap=gb[:, :nch, :],
                            in_ap=zfull.ap()[b * BROWS:(b + 1) * BROWS],
                            idxs_ap=idx_t[:, qs * 8:(qs + nch) * 8],
                            num_idxs=nch * 128, num_idxs_reg=nch * 128,
                            elem_size=64, single_packet=False,
                            queue_num=ci % NQ)
                        qoff = 0
                        while qoff < nch:
                            gsz = min(PBATCH, nch - qoff)
                            P = pbufp.tile([128, PBATCH * 128], f32, tag="P")
                            nc.vector.tensor_tensor(
                                out=P[:, :gsz * 128].rearrange("p (g v) -> p g v", g=gsz),
                                in0=dstrel_t[:, qs + qoff:qs + qoff + gsz]
                                    .to_broadcast([128, gsz, 128]),
                                in1=iota[:].rearrange("p (o v) -> p o v", o=1)
                                    .to_broadcast([128, gsz, 128]),
                                op=ALU.is_equal)
                            for j in range(gsz):
                                qq = qs + qoff + j
                                t = int(tile_of_chunk[qq])
                                ti = t - g[0]
                                if fm:
                                    # PSUM accumulation groups are per 2KB bank
                                    # (= 4 tiles of 128 fp32 cols): start/stop
                                    # only on the bank's first/last matmul.
                                    bft = g[0] + (ti // 4) * 4
                                    blt = min(bft + 3, g[-1])
                                else:
                                    bft, blt = g[0], g[-1]
                                first = (b == 0) and (t == bft) and (qq == chunk_start[bft, 0])
                                last = (b == NBUCK - 1) and (t == blt) and \
                                    (qq == chunk_start[blt, NBUCK - 1] + csched[blt, NBUCK - 1] - 1)
                                if fm:
                                    nc.tensor.matmul(
                                        ps[:, ti * 128:(ti + 1) * 128],
                                        gb[:, qoff + j, 0:Fw],
                                        P[:, j * 128:(j + 1) * 128],
                                        start=first, stop=last, skip_group_check=True)
                                else:
                                    nc.tensor.matmul(
                                        ps[:, ti * F3:(ti + 1) * F3],
                                        P[:, j * 128:(j + 1) * 128],
                                        gb[:, qoff + j, 0:F3],
                                        start=first, stop=last, skip_group_check=True)
                            qoff += gsz
                        if b == NBUCK - 1:
                            if fm:
                                rc_sl = slabp.tile([64, GW], f32, tag="rcsl")
                                nc.sync.dma_start(rc_sl[:Fw, :gw], t_rcnt_fm.ap()[:Fw, c0:c0 + gw])
                                zr_sl2 = slabp.tile([64, GW], f32, tag="zrsl2")
                                nc.sync.dma_start(zr_sl2[:Fw, :gw], zr_src[:, c0:c0 + gw])
                                hsl = stagep.tile([64, GW], f32, tag="hsl")
                                nc.vector.tensor_mul(hsl[:Fw, :gw], ps[:, :gw], rc_sl[:Fw, :gw])
                                nc.vector.tensor_add(hsl[:Fw, :gw], hsl[:Fw, :gw], zr_sl2[:Fw, :gw])
                                s_p = smallp.tile([Fw, 2], f32, tag=f"stat_{scope}_{gi}")
                                nc.vector.tensor_reduce(s_p[:, 0:1], hsl[:Fw, :gw],
                                                        axis=AX.X, op=ALU.add)
                                sq_scr = stagep.tile([64, GW], f32, tag="sqscr")
                                nc.scalar.activation(sq_scr[:Fw, :gw], hsl[:Fw, :gw],
                                                     ACT.Square, accum_out=s_p[:, 1:2])
                                stat_parts.append(s_p)
                                nc.sync.dma_start(h_sink.ap()[:, c0:c0 + gw], hsl[:Fw, :gw])
                            else:
                                zr_sl3 = slabp.tile([128, GSIZE * F3], f32, tag="zrsl3")
                                nc.sync.dma_start(
                                    zr_sl3[:, :len(g) * F3].rearrange(
                                        "p (t f) -> p t f", f=F3),
                                    zr_src.ap()[c0:c0 + gw].rearrange(
                                        "(t p) f -> p t f", p=128))
                                for ti, t in enumerate(g):
                                    final_cb(ps[:, ti * F3:(ti + 1) * F3],
                                             zr_sl3[:, ti * F3:(ti + 1) * F3], t)
                return stat_parts

            def bn_finalize(stat_parts, Fw, bn_in, bn_out, g_t, be_t, scope):
                with nc.named_scope(scope):
                    np_ = len(stat_parts)
                    stk = smallp.tile([Fw, 2 * np_], f32, tag=f"stk_{scope}")
                    for i, s_p in enumerate(stat_parts):
                        nc.vector.tensor_copy(stk[:, 2 * i:2 * i + 2], s_p[:])
                    tot = smallp.tile([Fw, 2], f32, tag=f"tot_{scope}")
                    v = stk[:].rearrange("f (i two) -> f two i", two=2)
                    nc.vector.tensor_reduce(tot[:, 0:1], v[:, 0:1, :], axis=AX.X, op=ALU.add)
                    nc.vector.tensor_reduce(tot[:, 1:2], v[:, 1:2, :], axis=AX.X, op=ALU.add)
                    nc.sync.dma_start(bn_in.ap(), tot[:])
                    nc.gpsimd.collective_compute(
                        "AllReduce", ALU.add, replica_groups=RG,
                        ins=[bn_in.ap()], outs=[bn_out.ap()])
                    red = smallp.tile([Fw, 2], f32, tag=f"red_{scope}")
                    nc.sync.dma_start(red[:], bn_out.ap())
                    mean = smallp.tile([Fw, 1], f32, tag=f"mean_{scope}")
                    nc.vector.tensor_scalar_mul(mean[:], red[:, 0:1], 1.0 / N)
                    ex2 = smallp.tile([Fw, 1], f32, tag=f"ex2_{scope}")
                    nc.vector.tensor_scalar_mul(ex2[:], red[:, 1:2], 1.0 / N)
                    var = smallp.tile([Fw, 1], f32, tag=f"var_{scope}")
                    nc.vector.tensor_mul(var[:], mean[:], mean[:])
                    nc.vector.tensor_sub(var[:], ex2[:], var[:])
                    nc.vector.tensor_scalar_add(var[:], var[:], EPS)
                    std = smallp.tile([Fw, 1], f32, tag=f"std_{scope}")
                    nc.scalar.sqrt(std[:], var[:])
                    rstd = smallp.tile([Fw, 1], f32, tag=f"rstd_{scope}")
                    nc.vector.reciprocal(rstd[:], std[:])
                    scal = smallp.tile([Fw, 1], f32, tag=f"scal_{scope}")
                    nc.vector.tensor_mul(scal[:], g_t[:], rstd[:])
                    shift = smallp.tile([Fw, 1], f32, tag=f"shift_{scope}")
                    nc.vector.tensor_mul(shift[:], mean[:], scal[:])
                    nc.vector.tensor_sub(shift[:], be_t[:], shift[:])
                    return scal, shift

            stats1 = agg_layer(zfull1, F1, True, zrT1_d.ap(), hT1_d, "L1agg")
            scal1, shift1 = bn_finalize(stats1, F1, bn_in1, bn_out1, g1_t, be1_t, "BN1")

            # ================= layer-2 z phase =================
            with nc.named_scope("L2z"):
                for gi, g in enumerate(groups):
                    gw = len(g) * 128
                    c0 = g[0] * 128
                    hs = slabp.tile([64, GW], f32, tag="hs")
                    nc.sync.dma_start(hs[:F1, :gw], hT1_d.ap()[:, c0:c0 + gw])
                    nc.scalar.activation(hs[:F1, :gw], hs[:F1, :gw], ACT.Relu,
                                         bias=shift1[:], scale=scal1[:])
                    if g[-1] == NT - 1:
                        nc.vector.memzero(hs[:F1, NPC - c0:gw])
                    zr_sl = stagep.tile([64, GW], f32, tag="zrslab")
                    for ti, t in enumerate(g):
                        hst = hs[:F1, ti * 128:(ti + 1) * 128]
                        pz = zpsum.tile([128, 128], f32, tag="zps")
                        nc.tensor.matmul(pz[:, :64], hst, W2l_t[:], start=True, stop=True)
                        zs = sm3p.tile([128, 64], f32, tag="zstage")
                        nc.scalar.copy(zs[:], pz[:, :64])
                        nc.sync.dma_start(shard2.ap()[t * 128:(t + 1) * 128], zs[:])
                        pr = zpsum.tile([128, 128], f32, tag="zps")
                        nc.tensor.matmul(pr[:F2, :], W2r_t[:], hst, start=True, stop=True)
                        nc.scalar.copy(zr_sl[:F2, ti * 128:(ti + 1) * 128], pr[:F2, :])
                    nc.sync.dma_start(zrT2_d.ap()[:, c0:c0 + gw], zr_sl[:F2, :gw])

            with nc.named_scope("AG2"):
                nc.gpsimd.collective_compute(
                    "AllGather", ALU.bypass, replica_groups=RG,
                    ins=[shard2.ap()], outs=[zfull2.ap()])

            stats2 = agg_layer(zfull2, F2, True, zrT2_d.ap(), hT2_d, "L2agg")
            scal2, shift2 = bn_finalize(stats2, F2, bn_in2, bn_out2, g2_t, be2_t, "BN2")

            # ================= layer-3 z phase =================
            with nc.named_scope("L3z"):
                for gi, g in enumerate(groups):
                    gw = len(g) * 128
                    c0 = g[0] * 128
                    hs = slabp.tile([64, GW], f32, tag="hs")
                    nc.sync.dma_start(hs[:F2, :gw], hT2_d.ap()[:, c0:c0 + gw])
                    nc.scalar.activation(hs[:F2, :gw], hs[:F2, :gw], ACT.Relu,
                                         bias=shift2[:], scale=scal2[:])
                    if g[-1] == NT - 1:
                        nc.vector.memzero(hs[:F2, NPC - c0:gw])
                    for ti, t in enumerate(g):
                        hst = hs[:F2, ti * 128:(ti + 1) * 128]
                        pz = zpsum.tile([128, 128], f32, tag="zps")
                        nc.tensor.matmul(pz[:, :64], hst, W3l_t[:], start=True, stop=True)
                        zs = sm3p.tile([128, 64], f32, tag="zstage")
                        nc.scalar.copy(zs[:], pz[:, :64])
                        nc.sync.dma_start(shard3.ap()[t * 128:(t + 1) * 128], zs[:])
                        pr = zpsum.tile([128, 128], f32, tag="zps")
                        nc.tensor.matmul(pr[:, :F3], hst, W3r_t[:], start=True, stop=True)
                        zs3 = sm3p.tile([128, F3], f32, tag="z3stage")
                        nc.scalar.copy(zs3[:], pr[:, :F3])
                        nc.sync.dma_start(zr3_d.ap()[t * 128:(t + 1) * 128], zs3[:])

            with nc.named_scope("AG3"):
                nc.gpsimd.collective_compute(
                    "AllGather", ALU.bypass, replica_groups=RG,
                    ins=[shard3.ap()], outs=[zfull3.ap()])

            def l3_final(ps_slice, zr_slice, t):
                h3 = sm3p.tile([128, F3], f32, tag="h3")
                nc.vector.tensor_scalar(
                    out=h3[:], in0=ps_slice, scalar1=rcnt_nm_t[:, t:t + 1],
                    scalar2=None, op0=ALU.mult)
                nc.vector.tensor_add(h3[:], h3[:], zr_slice)
                nc.vector.tensor_add(h3[:], h3[:], b3rep[:])
                mx = sm3p.tile([128, 1], f32, tag="mx")
                nc.vector.tensor_reduce(mx[:], h3[:], axis=AX.X, op=ALU.max)
                nc.vector.tensor_scalar(out=h3[:], in0=h3[:], scalar1=mx[:],
                                        scalar2=None, op0=ALU.subtract)
                ex = sm3p.tile([128, F3], f32, tag="ex")
                se = sm3p.tile([128, 1], f32, tag="se")
                nc.scalar.activation(ex[:], h3[:], ACT.Exp, accum_out=se[:])
                ls = sm3p.tile([128, 1], f32, tag="ls")
                nc.scalar.activation(ls[:], se[:], ACT.Ln)
                nc.vector.tensor_scalar(out=h3[:], in0=h3[:], scalar1=ls[:],
                                        scalar2=None, op0=ALU.subtract)
                nc.sync.dma_start(t_out.ap()[t * 128:(t + 1) * 128], h3[:])

            agg_layer(zfull3, F3, False, zr3_d, None, "L3agg", final_cb=l3_final)

    nc.compile()
    return nc


_PROG_CACHE = {}


def _in_maps(pp, inputs):
    x = np.asarray(inputs["x"], np.float32)
    iota = np.broadcast_to(np.arange(128, dtype=np.float32)[None, :], (128, 128)).copy()
    W2lp = np.zeros((F1, 64), np.float32)
    W2lp[:, :F2] = np.asarray(inputs["W2l"], np.float32)
    W3lp = np.zeros((F2, 64), np.float32)
    W3lp[:, :F3] = np.asarray(inputs["W3l"], np.float32)
    b3rep = np.broadcast_to(np.asarray(inputs["b3"], np.float32)[None, :], (128, F3)).copy()
    common = {
        "iota": iota,
        "W1l": np.asarray(inputs["W1l"], np.float32),
        "W1r": np.asarray(inputs["W1r"], np.float32),
        "W2lp": W2lp,
        "W2r": np.asarray(inputs["W2r"], np.float32),
        "W3lp": W3lp,
        "W3r": np.asarray(inputs["W3r"], np.float32),
        "g1": np.asarray(inputs["g1"], np.float32)[:, None].copy(),
        "be1": np.asarray(inputs["be1"], np.float32)[:, None].copy(),
        "g2": np.asarray(inputs["g2"], np.float32)[:, None].copy(),
        "be2": np.asarray(inputs["be2"], np.float32)[:, None].copy(),
        "b3rep": b3rep,
    }
    in_maps = []
    for c in range(NCORES):
        xT = np.zeros((FIN, NPAD), np.float32)
        xT[:, :NPC] = x[c * NPC:(c + 1) * NPC].T
        m = dict(common)
        m["xT"] = xT
        m["gidx"] = pp["idx_all"][c]
        m["dstrel"] = pp["dstrel_all"][c]
        m["rcnt_nm"] = pp["rcnt_nm"][c]
        m["rcnt_fm"] = np.broadcast_to(pp["rcnt_row"][c][None, :], (64, NPAD)).copy()
        in_maps.append(m)
    return in_maps


def kernel(**inputs):
    edge_index = np.asarray(inputs["edge_index"])
    pp = _preprocess(edge_index)
    key = (pp["nchunk"], pp["csched"].tobytes())
    if key not in _PROG_CACHE:
        _PROG_CACHE[key] = _build_program(pp)
    nc = _PROG_CACHE[key]
    in_maps = _in_maps(pp, inputs)
    from concourse.bass_utils import run_bass_kernel_spmd
    res = run_bass_kernel_spmd(nc, in_maps, core_ids=list(range(NCORES)))
    return np.concatenate([res.results[c]["out"][:NPC] for c in range(NCORES)], axis=0)



# revision 2
# speedup vs baseline: 1.3693x; 1.3693x over previous
"""GraphSAGE 3-layer GNN forward pass on 8 Trainium2 NeuronCores.

Sharding: nodes split by range across 8 cores (graph/data parallel).
Per layer the message table z = h @ Wl is computed shard-wise (node-major
rows) and AllGathered into a replicated DRAM table; each core aggregates
the edges whose dst is in its shard: dma_gather pulls z[src] rows (256B)
into SBUF and a one-hot matmul on the tensor engine does the segment-sum
into PSUM (feature-major for layers 1/2, node-major for layer 3).
Mean-normalization (1/deg), the self term h @ Wr, BatchNorm (stats
AllReduced), ReLU and log_softmax run on vector/scalar engines.
int16 gather indices only reach 32768 rows, so the table is processed in
4 buckets of 25088 rows with (dst-tile, bucket)-pure edge chunks.
"""

import math
import numpy as np

# ---------------- problem constants (hardcoded per contract) ----------------
N = 100000
E = 1600000
FIN = 200
NCORES = 8
NPC = N // NCORES            # 12500 nodes per core
NT = 98                      # dst tiles of 128 nodes per core
NPAD = NT * 128              # 12544
SHARD = NPAD                 # table rows contributed per core
TROWS = SHARD * NCORES       # 100352
NBUCK = 4
BROWS = TROWS // NBUCK       # 25088 (< 32768, int16-safe)
F1, F2, F3 = 64, 32, 17
EPS = 1e-5

# ---------------- tunables ----------------
NQ = 4                # SWDGE queues used for gather calls
NSQ = 4               # queues declared (allows NQ up to 4)
GSIZE = 8             # dst tiles per PSUM accumulation group
PBATCH = 8            # chunks per one-hot build DVE op


def _wrap16(idx_flat):
    """dma_gather index layout: position i -> partition i%16, col i//16,
    replicated across the 8 q7 core pairs (128 partitions)."""
    n = idx_flat.shape[0]
    w = idx_flat.reshape(n // 16, 16).T.copy()
    return np.tile(w, (8, 1))


def _preprocess(edge_index):
    src = np.asarray(edge_index[0], dtype=np.int64)
    dst = np.asarray(edge_index[1], dtype=np.int64)
    trow = (src // NPC) * SHARD + (src % NPC)   # global table row of src
    bucket = trow // BROWS
    rel = trow - bucket * BROWS

    dst_core = dst // NPC
    dloc = dst - dst_core * NPC
    tile_e = dloc >> 7
    dstrel_e = dloc & 127

    per_core = []
    needed = np.zeros((NCORES, NT, NBUCK), np.int64)
    for c in range(NCORES):
        m = dst_core == c
        key = tile_e[m] * NBUCK + bucket[m]
        order = np.argsort(key, kind="stable")
        cnts = np.bincount(key, minlength=NT * NBUCK).reshape(NT, NBUCK)
        per_core.append({
            "key": key[order],
            "rel": rel[m][order],
            "dstrel": dstrel_e[m][order],
            "cnt": np.bincount(dloc[m], minlength=NPC),
            "cnts": cnts,
        })
        needed[c] = (cnts + 127) >> 7
    csched = np.maximum(needed.max(axis=0), 1)   # [NT, NBUCK]

    groups = [list(range(g, min(g + GSIZE, NT))) for g in range(0, NT, GSIZE)]
    chunk_start = np.zeros((NT, NBUCK), np.int64)
    calls = []  # (bucket, chunk_qstart, nchunks, group_index)
    q = 0
    for gi, g in enumerate(groups):
        for b in range(NBUCK):
            nch = 0
            for t in g:
                chunk_start[t, b] = q + nch
                nch += int(csched[t, b])
            calls.append((b, q, nch, gi))
            q += nch
    nchunk = q
    tile_of_chunk = np.zeros(nchunk, np.int64)
    for t in range(NT):
        for b in range(NBUCK):
            s = chunk_start[t, b]
            tile_of_chunk[s:s + csched[t, b]] = t

    idx_all = np.zeros((NCORES, 128, nchunk * 8), np.int16)
    dstrel_all = np.full((NCORES, 128, nchunk), -1.0, np.float32)
    rcnt_nm = np.zeros((NCORES, 128, NT), np.float32)
    rcnt_row = np.zeros((NCORES, NPAD), np.float32)
    for c in range(NCORES):
        ck = per_core[c]
        seg_off = np.zeros(NT * NBUCK + 1, np.int64)
        seg_off[1:] = np.cumsum(ck["cnts"].reshape(-1))
        pos = np.arange(len(ck["key"])) - seg_off[ck["key"]]
        t_e = ck["key"] // NBUCK
        b_e = ck["key"] % NBUCK
        qg = chunk_start[t_e, b_e] + (pos >> 7)
        p = pos & 127
        idx_flat = np.zeros(nchunk * 128, np.int16)
        idx_flat[qg * 128 + p] = ck["rel"].astype(np.int16)
        idx_all[c] = _wrap16(idx_flat)
        dstrel_all[c][p, qg] = ck["dstrel"].astype(np.float32)
        rc_pad = np.ones(NPAD, np.float32)
        rc_pad[:NPC] = 1.0 / np.maximum(ck["cnt"], 1).astype(np.float32)
        rcnt_nm[c] = rc_pad.reshape(NT, 128).T
        rcnt_row[c] = rc_pad

    return {
        "csched": csched, "groups": groups, "calls": calls, "nchunk": nchunk,
        "chunk_start": chunk_start, "tile_of_chunk": tile_of_chunk,
        "idx_all": idx_all, "dstrel_all": dstrel_all,
        "rcnt_nm": rcnt_nm, "rcnt_row": rcnt_row,
    }


def _build_program(pp):
    import concourse.bacc as bacc
    import concourse.tile as tile
    import concourse.bass as bass
    import concourse.mybir as mybir

    f32 = mybir.dt.float32
    AX = mybir.AxisListType
    ALU = mybir.AluOpType
    ACT = mybir.ActivationFunctionType

    groups = pp["groups"]
    calls = pp["calls"]
    csched = pp["csched"]
    chunk_start = pp["chunk_start"]
    tile_of_chunk = pp["tile_of_chunk"]
    nchunk = pp["nchunk"]
    max_call_chunks = max(nc_ for (_, _, nc_, _) in calls)

    nc = bacc.Bacc("TRN2", target_bir_lowering=False, debug=False,
                   num_devices=NCORES, num_swdge_queues=NSQ)

    # ---------------- I/O ----------------
    t_xT = nc.dram_tensor("xT", [FIN, NPAD], f32, kind="ExternalInput")
    t_idx = nc.dram_tensor("gidx", [128, nchunk * 8], mybir.dt.int16, kind="ExternalInput")
    t_dstrel = nc.dram_tensor("dstrel", [128, nchunk], f32, kind="ExternalInput")
    t_rcnt_nm = nc.dram_tensor("rcnt_nm", [128, NT], f32, kind="ExternalInput")
    t_rcnt_fm = nc.dram_tensor("rcnt_fm", [64, NPAD], f32, kind="ExternalInput")
    t_iota = nc.dram_tensor("iota", [128, 128], f32, kind="ExternalInput")
    t_W1l = nc.dram_tensor("W1l", [FIN, F1], f32, kind="ExternalInput")
    t_W1r = nc.dram_tensor("W1r", [FIN, F1], f32, kind="ExternalInput")
    t_W2l = nc.dram_tensor("W2lp", [F1, 64], f32, kind="ExternalInput")
    t_W2r = nc.dram_tensor("W2r", [F1, F2], f32, kind="ExternalInput")
    t_W3l = nc.dram_tensor("W3lp", [F2, 64], f32, kind="ExternalInput")
    t_W3r = nc.dram_tensor("W3r", [F2, F3], f32, kind="ExternalInput")
    t_g1 = nc.dram_tensor("g1", [F1, 1], f32, kind="ExternalInput")
    t_be1 = nc.dram_tensor("be1", [F1, 1], f32, kind="ExternalInput")
    t_g2 = nc.dram_tensor("g2", [F2, 1], f32, kind="ExternalInput")
    t_be2 = nc.dram_tensor("be2", [F2, 1], f32, kind="ExternalInput")
    t_b3 = nc.dram_tensor("b3rep", [128, F3], f32, kind="ExternalInput")
    t_out = nc.dram_tensor("out", [NPAD, F3], f32, kind="ExternalOutput")

    shard1 = nc.dram_tensor("shard1", [SHARD, 64], f32, kind="Internal")
    shard2 = nc.dram_tensor("shard2", [SHARD, 64], f32, kind="Internal")
    shard3 = nc.dram_tensor("shard3", [SHARD, 64], f32, kind="Internal")
    zfull1 = nc.dram_tensor("zfull1", [TROWS, 64], f32, kind="Internal", addr_space="Shared")
    zfull2 = nc.dram_tensor("zfull2", [TROWS, 64], f32, kind="Internal", addr_space="Shared")
    zfull3 = nc.dram_tensor("zfull3", [TROWS, 64], f32, kind="Internal", addr_space="Shared")
    zrT1_d = nc.dram_tensor("zrT1", [64, NPAD], f32, kind="Internal")
    zrT2_d = nc.dram_tensor("zrT2", [F2, NPAD], f32, kind="Internal")
    zr3_d = nc.dram_tensor("zr3", [NPAD, F3], f32, kind="Internal")
    hT1_d = nc.dram_tensor("hT1", [64, NPAD], f32, kind="Internal")
    hT2_d = nc.dram_tensor("hT2", [F2, NPAD], f32, kind="Internal")
    bn_in1 = nc.dram_tensor("bn_in1", [F1, 2], f32, kind="Internal")
    bn_out1 = nc.dram_tensor("bn_out1", [F1, 2], f32, kind="Internal", addr_space="Shared")
    bn_in2 = nc.dram_tensor("bn_in2", [F2, 2], f32, kind="Internal")
    bn_out2 = nc.dram_tensor("bn_out2", [F2, 2], f32, kind="Internal", addr_space="Shared")

    RG = [list(range(NCORES))]
    GW = GSIZE * 128

    with tile.TileContext(nc) as tc:
        with tc.tile_pool(name="const", bufs=1) as constp, \
             tc.tile_pool(name="wpool", bufs=1) as wpool, \
             tc.tile_pool(name="stage", bufs=2) as stagep, \
             tc.tile_pool(name="sm3", bufs=3) as sm3p, \
             tc.tile_pool(name="slab", bufs=2) as slabp, \
             tc.tile_pool(name="gbuf", bufs=4) as gbufp, \
             tc.tile_pool(name="pbuf", bufs=3) as pbufp, \
             tc.tile_pool(name="zpsum", bufs=2, space="PSUM") as zpsum, \
             tc.tile_pool(name="spsum", bufs=2, space="PSUM") as spsum, \
             tc.tile_pool(name="small", bufs=1) as smallp:

            # ---- constants
            iota = constp.tile([128, 128], f32)
            nc.sync.dma_start(iota[:], t_iota.ap())
            idx_t = constp.tile([128, nchunk * 8], mybir.dt.int16)
            nc.sync.dma_start(idx_t[:], t_idx.ap())
            dstrel_t = constp.tile([128, nchunk], f32)
            nc.sync.dma_start(dstrel_t[:], t_dstrel.ap())
            rcnt_nm_t = constp.tile([128, NT], f32)
            nc.sync.dma_start(rcnt_nm_t[:], t_rcnt_nm.ap())
            b3rep = constp.tile([128, F3], f32)
            nc.sync.dma_start(b3rep[:], t_b3.ap())

            def wload(name, tt, shape):
                w = wpool.tile(shape, f32, tag=name)
                nc.sync.dma_start(w[:], tt)
                return w

            W1l_a = wload("w1la", t_W1l.ap()[:128], [128, F1])
            W1l_b = wload("w1lb", t_W1l.ap()[128:], [72, F1])
            W1r_a = wload("w1ra", t_W1r.ap()[:128], [128, F1])
            W1r_b = wload("w1rb", t_W1r.ap()[128:], [72, F1])
            W2l_t = wload("w2l", t_W2l.ap(), [F1, 64])
            W2r_t = wload("w2r", t_W2r.ap(), [F1, F2])
            W3l_t = wload("w3l", t_W3l.ap(), [F2, 64])
            W3r_t = wload("w3r", t_W3r.ap(), [F2, F3])
            g1_t = wload("g1", t_g1.ap(), [F1, 1])
            be1_t = wload("be1", t_be1.ap(), [F1, 1])
            g2_t = wload("g2", t_g2.ap(), [F2, 1])
            be2_t = wload("be2", t_be2.ap(), [F2, 1])

            # ================= layer-1 z phase =================
            with nc.named_scope("L1z"):
                for gi, g in enumerate(groups):
                    gw = len(g) * 128
                    c0 = g[0] * 128
                    xa = slabp.tile([128, GW], f32, tag="xa")
                    xb = slabp.tile([72, GW], f32, tag="xb")
                    nc.sync.dma_start(xa[:, :gw], t_xT.ap()[:128, c0:c0 + gw])
                    nc.sync.dma_start(xb[:, :gw], t_xT.ap()[128:, c0:c0 + gw])
                    zr_sl = stagep.tile([64, GW], f32, tag="zrslab")
                    for ti, t in enumerate(g):
                        xs_a = xa[:, ti * 128:(ti + 1) * 128]
                        xs_b = xb[:, ti * 128:(ti + 1) * 128]
                        pz = zpsum.tile([128, 128], f32, tag="zps")
                        nc.tensor.matmul(pz[:, :F1], xs_a, W1l_a[:], start=True, stop=False)
                        nc.tensor.matmul(pz[:, :F1], xs_b, W1l_b[:], start=False, stop=True)
                        zs = sm3p.tile([128, 64], f32, tag="zstage")
                        nc.scalar.copy(zs[:], pz[:, :F1])
                        nc.sync.dma_start(shard1.ap()[t * 128:(t + 1) * 128], zs[:])
                        pr = zpsum.tile([128, 128], f32, tag="zps")
                        nc.tensor.matmul(pr[:F1, :], W1r_a[:], xs_a, start=True, stop=False)
                        nc.tensor.matmul(pr[:F1, :], W1r_b[:], xs_b, start=False, stop=True)
                        nc.scalar.copy(zr_sl[:, ti * 128:(ti + 1) * 128], pr[:F1, :])
                    nc.sync.dma_start(zrT1_d.ap()[:, c0:c0 + gw], zr_sl[:, :gw])

            with nc.named_scope("AG1"):
                nc.gpsimd.collective_compute(
                    "AllGather", ALU.bypass, replica_groups=RG,
                    ins=[shard1.ap()], outs=[zfull1.ap()])

            # ========== generic gather/aggregate ==========
            def agg_layer(zfull, Fw, fm, zr_src, h_sink, scope, final_cb=None):
                stat_parts = []
                with nc.named_scope(scope):
                    cur_ps = None
                    for ci, (b, qs, nch, gi) in enumerate(calls):
                        g = groups[gi]
                        gw = len(g) * 128
                        c0 = g[0] * 128
                        if b == 0:
                            if fm:
                                cur_ps = spsum.tile([Fw, GW], f32, tag="sacc")
                            else:
                                cur_ps = spsum.tile([128, GSIZE * F3], f32, tag="sacc3")
                        ps = cur_ps
                        gb = gbufp.tile([128, max_call_chunks, 64], f32, tag="gb")
                        nc.gpsimd.dma_gather(
                            out_ap=gb[:, :nch, :],
                            in_ap=zfull.ap()[b * BROWS:(b + 1) * BROWS],
                            idxs_ap=idx_t[:, qs * 8:(qs + nch) * 8],
                            num_idxs=nch * 128, num_idxs_reg=nch * 128,
                            elem_size=64, single_packet=False,
                            queue_num=ci % NQ)
                        qoff = 0
                        while qoff < nch:
                            gsz = min(PBATCH, nch - qoff)
                            P = pbufp.tile([128, PBATCH * 128], f32, tag="P")
                            nc.vector.tensor_tensor(
                                out=P[:, :gsz * 128].rearrange("p (g v) -> p g v", g=gsz),
                                in0=dstrel_t[:, qs + qoff:qs + qoff + gsz]
                                    .to_broadcast([128, gsz, 128]),
                                in1=iota[:].rearrange("p (o v) -> p o v", o=1)
                                    .to_broadcast([128, gsz, 128]),
                                op=ALU.is_equal)
                            for j in range(gsz):
                                qq = qs + qoff + j
                                t = int(tile_of_chunk[qq])
                                ti = t - g[0]
                                if fm:
                                    # PSUM accumulation groups are per 2KB bank
                                    # (= 4 tiles of 128 fp32 cols): start/stop
                                    # only on the bank's first/last matmul.
                                    bft = g[0] + (ti // 4) * 4
                                    blt = min(bft + 3, g[-1])
                                else:
                                    bft, blt = g[0], g[-1]
                                first = (b == 0) and (t == bft) and (qq == chunk_start[bft, 0])
                                last = (b == NBUCK - 1) and (t == blt) and \
                                    (qq == chunk_start[blt, NBUCK - 1] + csched[blt, NBUCK - 1] - 1)
                                if fm:
                                    nc.tensor.matmul(
                                        ps[:, ti * 128:(ti + 1) * 128],
                                        gb[:, qoff + j, 0:Fw],
                                        P[:, j * 128:(j + 1) * 128],
                                        start=first, stop=last, skip_group_check=True)
                                else:
                                    nc.tensor.matmul(
                                        ps[:, ti * F3:(ti + 1) * F3],
                                        P[:, j * 128:(j + 1) * 128],
                                        gb[:, qoff + j, 0:F3],
                                        start=first, stop=last, skip_group_check=True)
                            qoff += gsz
                        if b == NBUCK - 1:
                            if fm:
                                rc_sl = slabp.tile([64, GW], f32, tag="rcsl")
                                nc.sync.dma_start(rc_sl[:Fw, :gw], t_rcnt_fm.ap()[:Fw, c0:c0 + gw])
                                zr_sl2 = slabp.tile([64, GW], f32, tag="zrsl2")
                                nc.sync.dma_start(zr_sl2[:Fw, :gw], zr_src[:, c0:c0 + gw])
                                hsl = stagep.tile([64, GW], f32, tag="hsl")
                                nc.vector.tensor_mul(hsl[:Fw, :gw], ps[:, :gw], rc_sl[:Fw, :gw])
                                nc.vector.tensor_add(hsl[:Fw, :gw], hsl[:Fw, :gw], zr_sl2[:Fw, :gw])
                                s_p = smallp.tile([Fw, 2], f32, tag=f"stat_{scope}_{gi}")
                                nc.vector.tensor_reduce(s_p[:, 0:1], hsl[:Fw, :gw],
                                                        axis=AX.X, op=ALU.add)
                                sq_scr = stagep.tile([64, GW], f32, tag="sqscr")
                                nc.scalar.activation(sq_scr[:Fw, :gw], hsl[:Fw, :gw],
                                                     ACT.Square, accum_out=s_p[:, 1:2])
                                stat_parts.append(s_p)
                                nc.sync.dma_start(h_sink.ap()[:, c0:c0 + gw], hsl[:Fw, :gw])
                            else:
                                zr_sl3 = slabp.tile([128, GSIZE * F3], f32, tag="zrsl3")
                                nc.sync.dma_start(
                                    zr_sl3[:, :len(g) * F3].rearrange(
                                        "p (t f) -> p t f", f=F3),
                                    zr_src.ap()[c0:c0 + gw].rearrange(
                                        "(t p) f -> p t f", p=128))
                                for ti, t in enumerate(g):
                                    final_cb(ps[:, ti * F3:(ti + 1) * F3],
                                             zr_sl3[:, ti * F3:(ti + 1) * F3], t)
                return stat_parts

            def bn_finalize(stat_parts, Fw, bn_in, bn_out, g_t, be_t, scope):
                with nc.named_scope(scope):
                    np_ = len(stat_parts)
                    stk = smallp.tile([Fw, 2 * np_], f32, tag=f"stk_{scope}")
                    for i, s_p in enumerate(stat_parts):
                        nc.vector.tensor_copy(stk[:, 2 * i:2 * i + 2], s_p[:])
                    tot = smallp.tile([Fw, 2], f32, tag=f"tot_{scope}")
                    v = stk[:].rearrange("f (i two) -> f two i", two=2)
                    nc.vector.tensor_reduce(tot[:, 0:1], v[:, 0:1, :], axis=AX.X, op=ALU.add)
                    nc.vector.tensor_reduce(tot[:, 1:2], v[:, 1:2, :], axis=AX.X, op=ALU.add)
                    nc.sync.dma_start(bn_in.ap(), tot[:])
                    nc.gpsimd.collective_compute(
                        "AllReduce", ALU.add, replica_groups=RG,
                        ins=[bn_in.ap()], outs=[bn_out.ap()])
                    red = smallp.tile([Fw, 2], f32, tag=f"red_{scope}")
                    nc.sync.dma_start(red[:], bn_out.ap())
                    mean = smallp.tile([Fw, 1], f32, tag=f"mean_{scope}")
                    nc.vector.tensor_scalar_mul(mean[:], red[:, 0:1], 1.0 / N)
                    ex2 = smallp.tile([Fw, 1], f32, tag=f"ex2_{scope}")
                    nc.vector.tensor_scalar_mul(ex2[:], red[:, 1:2], 1.0 / N)
                    var = smallp.tile([Fw, 1], f32, tag=f"var_{scope}")
                    nc.vector.tensor_mul(var[:], mean[:], mean[:])
                    nc.vector.tensor_sub(var[:], ex2[:], var[:])
                    nc.vector.tensor_scalar_add(var[:], var[:], EPS)
                    std = smallp.tile([Fw, 1], f32, tag=f"std_{scope}")
                    nc.scalar.sqrt(std[:], var[:])
                    rstd = smallp.tile([Fw, 1], f32, tag=f"rstd_{scope}")
                    nc.vector.reciprocal(rstd[:], std[:])
                    scal = smallp.tile([Fw, 1], f32, tag=f"scal_{scope}")
                    nc.vector.tensor_mul(scal[:], g_t[:], rstd[:])
                    shift = smallp.tile([Fw, 1], f32, tag=f"shift_{scope}")
                    nc.vector.tensor_mul(shift[:], mean[:], scal[:])
                    nc.vector.tensor_sub(shift[:], be_t[:], shift[:])
                    return scal, shift

            stats1 = agg_layer(zfull1, F1, True, zrT1_d.ap(), hT1_d, "L1agg")
            scal1, shift1 = bn_finalize(stats1, F1, bn_in1, bn_out1, g1_t, be1_t, "BN1")

            # ================= layer-2 z phase =================
            with nc.named_scope("L2z"):
                for gi, g in enumerate(groups):
                    gw = len(g) * 128
                    c0 = g[0] * 128
                    hs = slabp.tile([64, GW], f32, tag="hs")
                    nc.sync.dma_start(hs[:F1, :gw], hT1_d.ap()[:, c0:c0 + gw])
                    nc.scalar.activation(hs[:F1, :gw], hs[:F1, :gw], ACT.Relu,
                                         bias=shift1[:], scale=scal1[:])
                    if g[-1] == NT - 1:
                        nc.vector.memzero(hs[:F1, NPC - c0:gw])
                    zr_sl = stagep.tile([64, GW], f32, tag="zrslab")
                    for ti, t in enumerate(g):
                        hst = hs[:F1, ti * 128:(ti + 1) * 128]
                        pz = zpsum.tile([128, 128], f32, tag="zps")
                        nc.tensor.matmul(pz[:, :64], hst, W2l_t[:], start=True, stop=True)
                        zs = sm3p.tile([128, 64], f32, tag="zstage")
                        nc.scalar.copy(zs[:], pz[:, :64])
                        nc.sync.dma_start(shard2.ap()[t * 128:(t + 1) * 128], zs[:])
                        pr = zpsum.tile([128, 128], f32, tag="zps")
                        nc.tensor.matmul(pr[:F2, :], W2r_t[:], hst, start=True, stop=True)
                        nc.scalar.copy(zr_sl[:F2, ti * 128:(ti + 1) * 128], pr[:F2, :])
                    nc.sync.dma_start(zrT2_d.ap()[:, c0:c0 + gw], zr_sl[:F2, :gw])

            with nc.named_scope("AG2"):
                nc.gpsimd.collective_compute(
                    "AllGather", ALU.bypass, replica_groups=RG,
                    ins=[shard2.ap()], outs=[zfull2.ap()])

            stats2 = agg_layer(zfull2, F2, True, zrT2_d.ap(), hT2_d, "L2agg")
            scal2, shift2 = bn_finalize(stats2, F2, bn_in2, bn_out2, g2_t, be2_t, "BN2")

            # ================= layer-3 z phase =================
            with nc.named_scope("L3z"):
                for gi, g in enumerate(groups):
                    gw = len(g) * 128
                    c0 = g[0] * 128
                    hs = slabp.tile([64, GW], f32, tag="hs")
                    nc.sync.dma_start(hs[:F2, :gw], hT2_d.ap()[:, c0:c0 + gw])
                    nc.scalar.activation(hs[:F2, :gw], hs[:F2, :gw], ACT.Relu,
                                         bias=shift2[:], scale=scal2[:])
                    if g[-1] == NT - 1:
                        nc.vector.memzero(hs[:F2, NPC - c0:gw])
                    for ti, t in enumerate(g):
                        hst = hs[:F2, ti * 128:(ti + 1) * 128]
                        pz = zpsum.tile([128, 128], f32, tag="zps")
                        nc.tensor.matmul(pz[:, :64], hst, W3l_t[:], start=True, stop=True)
                        zs = sm3p.tile([128, 64], f32, tag="zstage")
                        nc.scalar.copy(zs[:], pz[:, :64])
                        nc.sync.dma_start(shard3.ap()[t * 128:(t + 1) * 128], zs[:])
                        pr = zpsum.tile([128, 128], f32, tag="zps")
                        nc.tensor.matmul(pr[:, :F3], hst, W3r_t[:], start=True, stop=True)
                        zs3 = sm3p.tile([128, F3], f32, tag="z3stage")
                        nc.scalar.copy(zs3[:], pr[:, :F3])
                        nc.sync.dma_start(zr3_d.ap()[t * 128:(t + 1) * 128], zs3[:])

            with nc.named_scope("AG3"):
                nc.gpsimd.collective_compute(
                    "AllGather", ALU.bypass, replica_groups=RG,
                    ins=[shard3.ap()], outs=[zfull3.ap()])

            def l3_final(ps_slice, zr_slice, t):
                h3 = sm3p.tile([128, F3], f32, tag="h3")
                nc.vector.tensor_scalar(
                    out=h3[:], in0=ps_slice, scalar1=rcnt_nm_t[:, t:t + 1],
                    scalar2=None, op0=ALU.mult)
                nc.vector.tensor_add(h3[:], h3[:], zr_slice)
                nc.vector.tensor_add(h3[:], h3[:], b3rep[:])
                mx = sm3p.tile([128, 1], f32, tag="mx")
                nc.vector.tensor_reduce(mx[:], h3[:], axis=AX.X, op=ALU.max)
                nc.vector.tensor_scalar(out=h3[:], in0=h3[:], scalar1=mx[:],
                                        scalar2=None, op0=ALU.subtract)
                ex = sm3p.tile([128, F3], f32, tag="ex")
                se = sm3p.tile([128, 1], f32, tag="se")
                nc.scalar.activation(ex[:], h3[:], ACT.Exp, accum_out=se[:])
                ls = sm3p.tile([128, 1], f32, tag="ls")
                nc.scalar.activation(ls[:], se[:], ACT.Ln)
                nc.vector.tensor_scalar(out=h3[:], in0=h3[:], scalar1=ls[:],
                                        scalar2=None, op0=ALU.subtract)
                nc.sync.dma_start(t_out.ap()[t * 128:(t + 1) * 128], h3[:])

            agg_layer(zfull3, F3, False, zr3_d, None, "L3agg", final_cb=l3_final)

    nc.compile()
    return nc


_PROG_CACHE = {}


def _in_maps(pp, inputs):
    x = np.asarray(inputs["x"], np.float32)
    iota = np.broadcast_to(np.arange(128, dtype=np.float32)[None, :], (128, 128)).copy()
    W2lp = np.zeros((F1, 64), np.float32)
    W2lp[:, :F2] = np.asarray(inputs["W2l"], np.float32)
    W3lp = np.zeros((F2, 64), np.float32)
    W3lp[:, :F3] = np.asarray(inputs["W3l"], np.float32)
    b3rep = np.broadcast_to(np.asarray(inputs["b3"], np.float32)[None, :], (128, F3)).copy()
    common = {
        "iota": iota,
        "W1l": np.asarray(inputs["W1l"], np.float32),
        "W1r": np.asarray(inputs["W1r"], np.float32),
        "W2lp": W2lp,
        "W2r": np.asarray(inputs["W2r"], np.float32),
        "W3lp": W3lp,
        "W3r": np.asarray(inputs["W3r"], np.float32),
        "g1": np.asarray(inputs["g1"], np.float32)[:, None].copy(),
        "be1": np.asarray(inputs["be1"], np.float32)[:, None].copy(),
        "g2": np.asarray(inputs["g2"], np.float32)[:, None].copy(),
        "be2": np.asarray(inputs["be2"], np.float32)[:, None].copy(),
        "b3rep": b3rep,
    }
    in_maps = []
    for c in range(NCORES):
        xT = np.zeros((FIN, NPAD), np.float32)
        xT[:, :NPC] = x[c * NPC:(c + 1) * NPC].T
        m = dict(common)
        m["xT"] = xT
        m["gidx"] = pp["idx_all"][c]
        m["dstrel"] = pp["dstrel_all"][c]
        m["rcnt_nm"] = pp["rcnt_nm"][c]
        m["rcnt_fm"] = np.broadcast_to(pp["rcnt_row"][c][None, :], (64, NPAD)).copy()
        in_maps.append(m)
    return in_maps


def kernel(**inputs):
    edge_index = np.asarray(inputs["edge_index"])
    pp = _preprocess(edge_index)
    key = (pp["nchunk"], pp["csched"].tobytes())
    if key not in _PROG_CACHE:
        _PROG_CACHE[key] = _build_program(pp)
    nc = _PROG_CACHE[key]
    in_maps = _in_maps(pp, inputs)
    from concourse.bass_utils import run_bass_kernel_spmd
    res = run_bass_kernel_spmd(nc, in_maps, core_ids=list(range(NCORES)))
    return np.concatenate([res.results[c]["out"][:NPC] for c in range(NCORES)], axis=0)



# revision 9
# speedup vs baseline: 3.2414x; 2.3673x over previous
"""GraphSAGE 3-layer GNN forward pass on 8 Trainium2 NeuronCores.

Sharding: nodes split by range across 8 cores (graph/data parallel).
Per layer the message table z = h @ Wl is computed shard-wise (bf16, rows
padded to 128 cols = 256B) and AllGathered into a replicated DRAM table;
each core aggregates the edges whose dst is in its shard: dma_gather pulls
z[src] rows (256B) into SBUF and a one-hot matmul on the tensor engine does
the segment-sum into PSUM (feature-major for layers 1/2, node-major for
layer 3). Mean-normalization (1/deg), the self term h @ Wr (fp32), BatchNorm
(stats AllReduced), ReLU and a batched log_softmax run on vector/scalar.

Edge schedule: per (group of GSIZE dst tiles, src bucket) call, edges are
sorted by dst tile and packed contiguously (chunks may straddle tiles; the
union-of-cores (chunk, tile) piece schedule drives per-piece one-hots whose
dstrel is -1 outside the piece). Trailing idx slots are 0 (gathered then
zeroed by the one-hot); contiguous packing keeps the pad at 3.8%.
int16 gather indices only reach 32768 rows, so the table is processed in
4 buckets of 25088 rows. Gather calls rotate over 4 SWDGE queues so the 4
Q7 core pairs overlap descriptor generation.
"""

import numpy as np

# ---------------- problem constants (hardcoded per contract) ----------------
N = 100000
E = 1600000
FIN = 200
NCORES = 8
NPC = N // NCORES            # 12500 nodes per core
NT = 98                      # dst tiles of 128 nodes per core
NPAD = NT * 128              # 12544
SHARD = NPAD                 # table rows contributed per core
TROWS = SHARD * NCORES       # 100352
NBUCK = 4
BROWS = TROWS // NBUCK       # 25088 (< 32768, int16-safe)
F1, F2, F3 = 64, 32, 17
EPS = 1e-5

# ---------------- tunables ----------------
NQ = 4                # SWDGE queues used for gather calls
NSQ = 4               # queues declared
GSIZE = 4             # dst tiles per PSUM accumulation group
PBATCH = 8            # pieces per one-hot build DVE op
GBUFS = 8             # gather buffer pool depth


def _bf16(x):
    import ml_dtypes
    return np.asarray(x).astype(ml_dtypes.bfloat16)


def _wrap16(idx_flat):
    """dma_gather index layout: position i -> partition i%16, col i//16,
    replicated across the 8 q7 core pairs (128 partitions)."""
    n = idx_flat.shape[0]
    w = idx_flat.reshape(n // 16, 16).T.copy()
    return np.tile(w, (8, 1))


def _preprocess(edge_index):
    src = np.asarray(edge_index[0], dtype=np.int64)
    dst = np.asarray(edge_index[1], dtype=np.int64)
    trow = (src // NPC) * SHARD + (src % NPC)   # global table row of src
    bucket = trow // BROWS
    rel = trow - bucket * BROWS

    dst_core = dst // NPC
    dloc = dst - dst_core * NPC
    tile_e = dloc >> 7
    dstrel_e = dloc & 127

    groups = [list(range(g, min(g + GSIZE, NT))) for g in range(0, NT, GSIZE)]
    ngroups = len(groups)
    gi_of_tile = np.zeros(NT, np.int64)
    for gi, g in enumerate(groups):
        for t in g:
            gi_of_tile[t] = gi
    ncalls = ngroups * NBUCK

    # per-core, per-call edge lists sorted by tile
    per_core = []
    cnt = np.zeros((NCORES, ncalls), np.int64)
    for c in range(NCORES):
        m = dst_core == c
        gi_e = gi_of_tile[tile_e[m]]
        call_e = gi_e * NBUCK + bucket[m]
        # sort by (call, tile)
        order = np.lexsort((tile_e[m], call_e))
        per_core.append({
            "call": call_e[order],
            "tile": tile_e[m][order],
            "rel": rel[m][order],
            "dstrel": dstrel_e[m][order],
            "cnt_node": np.bincount(dloc[m], minlength=NPC),
        })
        cnt[c] = np.bincount(call_e, minlength=ncalls)

    nch_call = np.maximum((cnt.max(axis=0) + 127) // 128, 1)   # [ncalls]

    # union piece schedule: per call, sorted set of (chunk, tile)
    pieces_per_call = []
    for ci in range(ncalls):
        pieces = set()
        for c in range(NCORES):
            ck = per_core[c]
            sel = np.nonzero(ck["call"] == ci)[0]
            if len(sel) == 0:
                continue
            pos = np.arange(len(sel))
            chunks = pos >> 7
            tiles = ck["tile"][sel]
            pieces.update(zip(chunks.tolist(), tiles.tolist()))
        gi = ci // NBUCK
        if not pieces:
            pieces = {(0, groups[gi][0])}
        pieces_per_call.append(sorted(pieces))

    # flat piece arrays + start/stop flags (one PSUM bank per group; the
    # accumulation group spans the 4 bucket-calls of a group)
    piece_chunk, piece_tile = [], []
    piece_start, piece_stop = [], []
    call_pstart = np.zeros(ncalls + 1, np.int64)
    for gi in range(ngroups):
        for b in range(NBUCK):
            ci = gi * NBUCK + b
            call_pstart[ci] = len(piece_chunk)
            for k, (ch, t) in enumerate(pieces_per_call[ci]):
                piece_chunk.append(ch)
                piece_tile.append(t)
                piece_start.append(b == 0 and k == 0)
                piece_stop.append(b == NBUCK - 1 and k == len(pieces_per_call[ci]) - 1)
    call_pstart[ncalls] = len(piece_chunk)
    npieces = len(piece_chunk)

    # per-core idx (wrapped, trailing -1) and per-piece dstrel planes
    idx_cols = int(nch_call.sum()) * 8
    idx_all = np.zeros((NCORES, 128, idx_cols), np.int16)
    dstrel_all = np.full((NCORES, 128, npieces), -1.0, np.float32)
    rcnt_row = np.zeros((NCORES, NPAD), np.float32)
    rcnt_nm = np.zeros((NCORES, 128, NT), np.float32)
    for c in range(NCORES):
        ck = per_core[c]
        coff = 0
        idx_parts = []
        for ci in range(ncalls):
            sel = np.nonzero(ck["call"] == ci)[0]
            nidx = int(nch_call[ci]) * 128
            flat = np.zeros(nidx, np.int16)
            flat[:len(sel)] = ck["rel"][sel].astype(np.int16)
            idx_parts.append(_wrap16(flat))
            # per-piece dstrel
            pos = np.arange(len(sel))
            chunks = pos >> 7
            lanes = pos & 127
            tiles = ck["tile"][sel]
            p0 = call_pstart[ci]
            pmap = {(ch, t): k for k, (ch, t) in enumerate(pieces_per_call[ci])}
            pidx = np.fromiter((pmap[(ch, t)] for ch, t in zip(chunks.tolist(), tiles.tolist())),
                               np.int64, count=len(sel))
            dstrel_all[c][lanes, p0 + pidx] = ck["dstrel"][sel].astype(np.float32)
            coff += nidx
        idx_all[c] = np.concatenate(idx_parts, axis=1)
        rc_pad = np.ones(NPAD, np.float32)
        rc_pad[:NPC] = 1.0 / np.maximum(ck["cnt_node"], 1).astype(np.float32)
        rcnt_row[c] = rc_pad
        rcnt_nm[c] = rc_pad.reshape(NT, 128).T

    return {
        "groups": groups, "ncalls": ncalls, "nch_call": nch_call,
        "call_pstart": call_pstart, "npieces": npieces,
        "piece_chunk": np.array(piece_chunk), "piece_tile": np.array(piece_tile),
        "piece_start": np.array(piece_start), "piece_stop": np.array(piece_stop),
        "idx_all": idx_all, "idx_cols": idx_cols, "dstrel_all": dstrel_all,
        "rcnt_row": rcnt_row, "rcnt_nm": rcnt_nm,
    }


def _build_program(pp):
    import concourse.bacc as bacc
    import concourse.tile as tile
    import concourse.bass as bass
    import concourse.mybir as mybir

    f32 = mybir.dt.float32
    bf16 = mybir.dt.bfloat16
    AX = mybir.AxisListType
    ALU = mybir.AluOpType
    ACT = mybir.ActivationFunctionType

    groups = pp["groups"]
    ngroups = len(groups)
    ncalls = pp["ncalls"]
    nch_call = pp["nch_call"]
    call_pstart = pp["call_pstart"]
    npieces = pp["npieces"]
    piece_chunk = pp["piece_chunk"]
    piece_tile = pp["piece_tile"]
    piece_start = pp["piece_start"]
    piece_stop = pp["piece_stop"]
    idx_cols = pp["idx_cols"]
    MCC = int(nch_call.max())
    GW = GSIZE * 128

    nc = bacc.Bacc("TRN2", target_bir_lowering=False, debug=False,
                   num_devices=NCORES, num_swdge_queues=NSQ)

    # ---------------- I/O ----------------
    t_xT = nc.dram_tensor("xT", [FIN, NPAD], bf16, kind="ExternalInput")
    t_idx = nc.dram_tensor("gidx", [128, idx_cols], mybir.dt.int16, kind="ExternalInput")
    t_dstrel = nc.dram_tensor("dstrel", [128, npieces], bf16, kind="ExternalInput")
    t_rcnt_nm = nc.dram_tensor("rcnt_nm", [128, NT], f32, kind="ExternalInput")
    t_rcnt_fm = nc.dram_tensor("rcnt_fm", [64, NPAD], f32, kind="ExternalInput")
    t_iota = nc.dram_tensor("iota", [128, 128], bf16, kind="ExternalInput")
    t_W1l = nc.dram_tensor("W1l", [FIN, F1], bf16, kind="ExternalInput")
    t_W1r = nc.dram_tensor("W1r", [FIN, F1], bf16, kind="ExternalInput")
    t_W2l = nc.dram_tensor("W2l", [F1, F2], bf16, kind="ExternalInput")
    t_W2r = nc.dram_tensor("W2r", [F1, F2], bf16, kind="ExternalInput")
    t_W3l = nc.dram_tensor("W3l", [F2, F3], bf16, kind="ExternalInput")
    t_W3r = nc.dram_tensor("W3r", [F2, F3], bf16, kind="ExternalInput")
    t_g1 = nc.dram_tensor("g1", [F1, 1], f32, kind="ExternalInput")
    t_be1 = nc.dram_tensor("be1", [F1, 1], f32, kind="ExternalInput")
    t_g2 = nc.dram_tensor("g2", [F2, 1], f32, kind="ExternalInput")
    t_be2 = nc.dram_tensor("be2", [F2, 1], f32, kind="ExternalInput")
    t_b3 = nc.dram_tensor("b3rep", [128, F3], f32, kind="ExternalInput")
    t_out = nc.dram_tensor("out", [NPAD, F3], f32, kind="ExternalOutput")

    shard1 = nc.dram_tensor("shard1", [SHARD, 128], bf16, kind="Internal")
    shard2 = nc.dram_tensor("shard2", [SHARD, 128], bf16, kind="Internal")
    shard3 = nc.dram_tensor("shard3", [SHARD, 128], bf16, kind="Internal")
    zfull1 = nc.dram_tensor("zfull1", [TROWS, 128], bf16, kind="Internal", addr_space="Shared")
    zfull2 = nc.dram_tensor("zfull2", [TROWS, 128], bf16, kind="Internal", addr_space="Shared")
    zfull3 = nc.dram_tensor("zfull3", [TROWS, 128], bf16, kind="Internal", addr_space="Shared")
    zrT1_d = nc.dram_tensor("zrT1", [64, NPAD], f32, kind="Internal")
    zrT2_d = nc.dram_tensor("zrT2", [F2, NPAD], f32, kind="Internal")
    zr3_d = nc.dram_tensor("zr3", [NPAD, F3], f32, kind="Internal")
    hT1_d = nc.dram_tensor("hT1", [64, NPAD], f32, kind="Internal")
    hT2_d = nc.dram_tensor("hT2", [F2, NPAD], f32, kind="Internal")
    bn_in1 = nc.dram_tensor("bn_in1", [F1, 2], f32, kind="Internal")
    bn_out1 = nc.dram_tensor("bn_out1", [F1, 2], f32, kind="Internal", addr_space="Shared")
    bn_in2 = nc.dram_tensor("bn_in2", [F2, 2], f32, kind="Internal")
    bn_out2 = nc.dram_tensor("bn_out2", [F2, 2], f32, kind="Internal", addr_space="Shared")

    RG = [list(range(NCORES))]

    with tile.TileContext(nc) as tc:
        with tc.tile_pool(name="const", bufs=1) as constp, \
             tc.tile_pool(name="wpool", bufs=1) as wpool, \
             tc.tile_pool(name="stage", bufs=2) as stagep, \
             tc.tile_pool(name="sm3", bufs=3) as sm3p, \
             tc.tile_pool(name="slab", bufs=2) as slabp, \
             tc.tile_pool(name="gbuf", bufs=GBUFS) as gbufp, \
             tc.tile_pool(name="pbuf", bufs=3) as pbufp, \
             tc.tile_pool(name="zpsum", bufs=2, space="PSUM") as zpsum, \
             tc.tile_pool(name="spsum", bufs=2, space="PSUM") as spsum, \
             tc.tile_pool(name="small", bufs=1) as smallp:

            # ---- constants
            iota = constp.tile([128, 128], bf16)
            nc.sync.dma_start(iota[:], t_iota.ap())
            idx_t = constp.tile([128, idx_cols], mybir.dt.int16)
            nc.sync.dma_start(idx_t[:], t_idx.ap())
            dstrel_t = constp.tile([128, npieces], bf16)
            nc.sync.dma_start(dstrel_t[:], t_dstrel.ap())
            rcnt_nm_t = constp.tile([128, NT], f32)
            nc.sync.dma_start(rcnt_nm_t[:], t_rcnt_nm.ap())
            b3rep = constp.tile([128, F3], f32)
            nc.sync.dma_start(b3rep[:], t_b3.ap())

            def wload(name, tt, shape, dt=bf16):
                w = wpool.tile(shape, dt, tag=name)
                nc.sync.dma_start(w[:], tt)
                return w

            W1l_a = wload("w1la", t_W1l.ap()[:128], [128, F1])
            W1l_b = wload("w1lb", t_W1l.ap()[128:], [72, F1])
            W1r_a = wload("w1ra", t_W1r.ap()[:128], [128, F1])
            W1r_b = wload("w1rb", t_W1r.ap()[128:], [72, F1])
            W2l_t = wload("w2l", t_W2l.ap(), [F1, F2])
            W2r_t = wload("w2r", t_W2r.ap(), [F1, F2])
            W3l_t = wload("w3l", t_W3l.ap(), [F2, F3])
            W3r_t = wload("w3r", t_W3r.ap(), [F2, F3])
            g1_t = wload("g1", t_g1.ap(), [F1, 1], f32)
            be1_t = wload("be1", t_be1.ap(), [F1, 1], f32)
            g2_t = wload("g2", t_g2.ap(), [F2, 1], f32)
            be2_t = wload("be2", t_be2.ap(), [F2, 1], f32)

            # pre-zero rotating pools whose stale regions reach matmuls
            for _ in range(GBUFS):
                gz = gbufp.tile([128, MCC, 128], bf16, tag="gb")
                nc.vector.memzero(gz[:])
            for _ in range(3):
                zz = sm3p.tile([128, 128], bf16, tag="zstage")
                nc.vector.memzero(zz[:])

            # ================= layer-1 z phase =================
            with nc.named_scope("L1z"):
                for gi, g in enumerate(groups):
                    gw = len(g) * 128
                    c0 = g[0] * 128
                    xa = slabp.tile([128, GW], bf16, tag="xa")
                    xb = slabp.tile([72, GW], bf16, tag="xb")
                    nc.sync.dma_start(xa[:, :gw], t_xT.ap()[:128, c0:c0 + gw])
                    nc.sync.dma_start(xb[:, :gw], t_xT.ap()[128:, c0:c0 + gw])
                    zr_sl = stagep.tile([64, GW], f32, tag="zrslab")
                    for ti, t in enumerate(g):
                        xs_a = xa[:, ti * 128:(ti + 1) * 128]
                        xs_b = xb[:, ti * 128:(ti + 1) * 128]
                        pz = zpsum.tile([128, 128], f32, tag="zps")
                        nc.tensor.matmul(pz[:, :F1], xs_a, W1l_a[:], start=True, stop=False)
                        nc.tensor.matmul(pz[:, :F1], xs_b, W1l_b[:], start=False, stop=True)
                        zs = sm3p.tile([128, 128], bf16, tag="zstage")
                        nc.scalar.copy(zs[:, 0:F1], pz[:, :F1])
                        nc.sync.dma_start(shard1.ap()[t * 128:(t + 1) * 128], zs[:])
                        pr = zpsum.tile([128, 128], f32, tag="zps")
                        nc.tensor.matmul(pr[:F1, :], W1r_a[:], xs_a, start=True, stop=False)
                        nc.tensor.matmul(pr[:F1, :], W1r_b[:], xs_b, start=False, stop=True)
                        nc.scalar.copy(zr_sl[:, ti * 128:(ti + 1) * 128], pr[:F1, :])
                    nc.sync.dma_start(zrT1_d.ap()[:, c0:c0 + gw], zr_sl[:, :gw])

            with nc.named_scope("AG1"):
                nc.gpsimd.collective_compute(
                    "AllGather", ALU.bypass, replica_groups=RG,
                    ins=[shard1.ap()], outs=[zfull1.ap()])

            # ========== generic gather/aggregate ==========
            def agg_layer(zfull, Fw, fm, zr_src, h_sink, scope, final_cb=None):
                stat_parts = []
                with nc.named_scope(scope):
                    cur_ps = None
                    for ci in range(ncalls):
                        gi, b = ci // NBUCK, ci % NBUCK
                        g = groups[gi]
                        gw = len(g) * 128
                        c0 = g[0] * 128
                        nch = int(nch_call[ci])
                        qs8 = int(nch_call[:ci].sum()) * 8
                        if b == 0:
                            if fm:
                                cur_ps = spsum.tile([Fw, GW], f32, tag="sacc")
                            else:
                                cur_ps = spsum.tile([128, GSIZE * F3], f32, tag="sacc3")
                        ps = cur_ps
                        gb = gbufp.tile([128, MCC, 128], bf16, tag="gb")
                        nc.gpsimd.dma_gather(
                            out_ap=gb[:, :nch, :],
                            in_ap=zfull.ap()[b * BROWS:(b + 1) * BROWS],
                            idxs_ap=idx_t[:, qs8:qs8 + nch * 8],
                            num_idxs=nch * 128, num_idxs_reg=nch * 128,
                            elem_size=128, single_packet=False,
                            queue_num=ci % NQ)
                        p0 = int(call_pstart[ci])
                        pend = int(call_pstart[ci + 1])
                        poff = p0
                        while poff < pend:
                            bs = min(PBATCH, pend - poff)
                            P = pbufp.tile([128, PBATCH * 128], bf16, tag="P")
                            nc.vector.tensor_tensor(
                                out=P[:, :bs * 128].rearrange("p (g v) -> p g v", g=bs),
                                in0=dstrel_t[:, poff:poff + bs]
                                    .to_broadcast([128, bs, 128]),
                                in1=iota[:].rearrange("p (o v) -> p o v", o=1)
                                    .to_broadcast([128, bs, 128]),
                                op=ALU.is_equal)
                            for j in range(bs):
                                pj = poff + j
                                ch = int(piece_chunk[pj])
                                t = int(piece_tile[pj])
                                ti = t - g[0]
                                first = bool(piece_start[pj])
                                last = bool(piece_stop[pj])
                                if fm:
                                    nc.tensor.matmul(
                                        ps[:, ti * 128:(ti + 1) * 128],
                                        gb[:, ch, 0:Fw],
                                        P[:, j * 128:(j + 1) * 128],
                                        start=first, stop=last, skip_group_check=True)
                                else:
                                    nc.tensor.matmul(
                                        ps[:, ti * F3:(ti + 1) * F3],
                                        P[:, j * 128:(j + 1) * 128],
                                        gb[:, ch, 0:F3],
                                        start=first, stop=last, skip_group_check=True)
                            poff += bs
                        if b == NBUCK - 1:
                            if fm:
                                rc_sl = slabp.tile([64, GW], f32, tag="rcsl")
                                nc.sync.dma_start(rc_sl[:Fw, :gw], t_rcnt_fm.ap()[:Fw, c0:c0 + gw])
                                zr_sl2 = slabp.tile([64, GW], f32, tag="zrsl2")
                                nc.sync.dma_start(zr_sl2[:Fw, :gw], zr_src[:, c0:c0 + gw])
                                hsl = stagep.tile([64, GW], f32, tag="hsl")
                                nc.vector.tensor_mul(hsl[:Fw, :gw], ps[:, :gw], rc_sl[:Fw, :gw])
                                nc.vector.tensor_add(hsl[:Fw, :gw], hsl[:Fw, :gw], zr_sl2[:Fw, :gw])
                                s_p = smallp.tile([Fw, 2], f32, tag=f"stat_{scope}_{gi}")
                                nc.vector.tensor_reduce(s_p[:, 0:1], hsl[:Fw, :gw],
                                                        axis=AX.X, op=ALU.add)
                                sq_scr = stagep.tile([64, GW], f32, tag="sqscr")
                                nc.scalar.activation(sq_scr[:Fw, :gw], hsl[:Fw, :gw],
                                                     ACT.Square, accum_out=s_p[:, 1:2])
                                stat_parts.append(s_p)
                                nc.sync.dma_start(h_sink.ap()[:, c0:c0 + gw], hsl[:Fw, :gw])
                            else:
                                final_cb(ps, g, gw, c0)
                return stat_parts

            def bn_finalize(stat_parts, Fw, bn_in, bn_out, g_t, be_t, scope):
                with nc.named_scope(scope):
                    np_ = len(stat_parts)
                    stk = smallp.tile([Fw, 2 * np_], f32, tag=f"stk_{scope}")
                    for i, s_p in enumerate(stat_parts):
                        nc.vector.tensor_copy(stk[:, 2 * i:2 * i + 2], s_p[:])
                    tot = smallp.tile([Fw, 2], f32, tag=f"tot_{scope}")
                    v = stk[:].rearrange("f (i two) -> f two i", two=2)
                    nc.vector.tensor_reduce(tot[:, 0:1], v[:, 0:1, :], axis=AX.X, op=ALU.add)
                    nc.vector.tensor_reduce(tot[:, 1:2], v[:, 1:2, :], axis=AX.X, op=ALU.add)
                    nc.sync.dma_start(bn_in.ap(), tot[:])
                    nc.gpsimd.collective_compute(
                        "AllReduce", ALU.add, replica_groups=RG,
                        ins=[bn_in.ap()], outs=[bn_out.ap()])
                    red = smallp.tile([Fw, 2], f32, tag=f"red_{scope}")
                    nc.sync.dma_start(red[:], bn_out.ap())
                    mean = smallp.tile([Fw, 1], f32, tag=f"mean_{scope}")
                    nc.vector.tensor_scalar_mul(mean[:], red[:, 0:1], 1.0 / N)
                    ex2 = smallp.tile([Fw, 1], f32, tag=f"ex2_{scope}")
                    nc.vector.tensor_scalar_mul(ex2[:], red[:, 1:2], 1.0 / N)
                    var = smallp.tile([Fw, 1], f32, tag=f"var_{scope}")
                    nc.vector.tensor_mul(var[:], mean[:], mean[:])
                    nc.vector.tensor_sub(var[:], ex2[:], var[:])
                    nc.vector.tensor_scalar_add(var[:], var[:], EPS)
                    std = smallp.tile([Fw, 1], f32, tag=f"std_{scope}")
                    nc.scalar.sqrt(std[:], var[:])
                    rstd = smallp.tile([Fw, 1], f32, tag=f"rstd_{scope}")
                    nc.vector.reciprocal(rstd[:], std[:])
                    scal = smallp.tile([Fw, 1], f32, tag=f"scal_{scope}")
                    nc.vector.tensor_mul(scal[:], g_t[:], rstd[:])
                    shift = smallp.tile([Fw, 1], f32, tag=f"shift_{scope}")
                    nc.vector.tensor_mul(shift[:], mean[:], scal[:])
                    nc.vector.tensor_sub(shift[:], be_t[:], shift[:])
                    return scal, shift

            stats1 = agg_layer(zfull1, F1, True, zrT1_d.ap(), hT1_d, "L1agg")
            scal1, shift1 = bn_finalize(stats1, F1, bn_in1, bn_out1, g1_t, be1_t, "BN1")

            # ================= layer-2 z phase =================
            with nc.named_scope("L2z"):
                for gi, g in enumerate(groups):
                    gw = len(g) * 128
                    c0 = g[0] * 128
                    hs = slabp.tile([64, GW], f32, tag="hs")
                    nc.sync.dma_start(hs[:F1, :gw], hT1_d.ap()[:, c0:c0 + gw])
                    hsb = slabp.tile([64, GW], bf16, tag="hsb")
                    nc.scalar.activation(hsb[:F1, :gw], hs[:F1, :gw], ACT.Relu,
                                         bias=shift1[:], scale=scal1[:])
                    if g[-1] == NT - 1:
                        nc.vector.memzero(hsb[:F1, NPC - c0:gw])
                    zr_sl = stagep.tile([64, GW], f32, tag="zrslab")
                    for ti, t in enumerate(g):
                        hst = hsb[:F1, ti * 128:(ti + 1) * 128]
                        pz = zpsum.tile([128, 128], f32, tag="zps")
                        nc.tensor.matmul(pz[:, :F2], hst, W2l_t[:], start=True, stop=True)
                        zs = sm3p.tile([128, 128], bf16, tag="zstage")
                        nc.scalar.copy(zs[:, 0:F2], pz[:, :F2])
                        nc.sync.dma_start(shard2.ap()[t * 128:(t + 1) * 128], zs[:])
                        pr = zpsum.tile([128, 128], f32, tag="zps")
                        nc.tensor.matmul(pr[:F2, :], W2r_t[:], hst, start=True, stop=True)
                        nc.scalar.copy(zr_sl[:F2, ti * 128:(ti + 1) * 128], pr[:F2, :])
                    nc.sync.dma_start(zrT2_d.ap()[:, c0:c0 + gw], zr_sl[:F2, :gw])

            with nc.named_scope("AG2"):
                nc.gpsimd.collective_compute(
                    "AllGather", ALU.bypass, replica_groups=RG,
                    ins=[shard2.ap()], outs=[zfull2.ap()])

            stats2 = agg_layer(zfull2, F2, True, zrT2_d.ap(), hT2_d, "L2agg")
            scal2, shift2 = bn_finalize(stats2, F2, bn_in2, bn_out2, g2_t, be2_t, "BN2")

            # ================= layer-3 z phase =================
            with nc.named_scope("L3z"):
                for gi, g in enumerate(groups):
                    gw = len(g) * 128
                    c0 = g[0] * 128
                    hs = slabp.tile([64, GW], f32, tag="hs")
                    nc.sync.dma_start(hs[:F2, :gw], hT2_d.ap()[:, c0:c0 + gw])
                    hsb = slabp.tile([64, GW], bf16, tag="hsb")
                    nc.scalar.activation(hsb[:F2, :gw], hs[:F2, :gw], ACT.Relu,
                                         bias=shift2[:], scale=scal2[:])
                    if g[-1] == NT - 1:
                        nc.vector.memzero(hsb[:F2, NPC - c0:gw])
                    for ti, t in enumerate(g):
                        hst = hsb[:F2, ti * 128:(ti + 1) * 128]
                        pz = zpsum.tile([128, 128], f32, tag="zps")
                        nc.tensor.matmul(pz[:, :F3], hst, W3l_t[:], start=True, stop=True)
                        zs = sm3p.tile([128, 128], bf16, tag="zstage")
                        nc.scalar.copy(zs[:, 0:F3], pz[:, :F3])
                        nc.sync.dma_start(shard3.ap()[t * 128:(t + 1) * 128], zs[:])
                        pr = zpsum.tile([128, 128], f32, tag="zps")
                        nc.tensor.matmul(pr[:, :F3], hst, W3r_t[:], start=True, stop=True)
                        zs3 = sm3p.tile([128, F3], f32, tag="z3stage")
                        nc.scalar.copy(zs3[:], pr[:, :F3])
                        nc.sync.dma_start(zr3_d.ap()[t * 128:(t + 1) * 128], zs3[:])

            with nc.named_scope("AG3"):
                nc.gpsimd.collective_compute(
                    "AllGather", ALU.bypass, replica_groups=RG,
                    ins=[shard3.ap()], outs=[zfull3.ap()])

            def l3_final(ps, g, gw, c0):
                ng = len(g)
                W = ng * F3
                zr_sl3 = slabp.tile([128, GSIZE * F3], f32, tag="zrsl3")
                nc.sync.dma_start(
                    zr_sl3[:, :W].rearrange("p (t f) -> p t f", f=F3),
                    zr3_d.ap()[c0:c0 + gw].rearrange("(t p) f -> p t f", p=128))
                h3 = sm3p.tile([128, GSIZE * F3], f32, tag="h3")
                nc.vector.tensor_tensor(
                    out=h3[:, :W].rearrange("p (t f) -> p t f", f=F3),
                    in0=ps[:, :W].rearrange("p (t f) -> p t f", f=F3),
                    in1=rcnt_nm_t[:, g[0]:g[0] + ng]
                        .rearrange("p (t o) -> p t o", o=1)
                        .to_broadcast([128, ng, F3]),
                    op=ALU.mult)
                nc.vector.tensor_add(h3[:, :W], h3[:, :W], zr_sl3[:, :W])
                nc.vector.tensor_tensor(
                    out=h3[:, :W].rearrange("p (t f) -> p t f", f=F3),
                    in0=h3[:, :W].rearrange("p (t f) -> p t f", f=F3),
                    in1=b3rep[:, :F3].rearrange("p (o f) -> p o f", o=1)
                        .to_broadcast([128, ng, F3]),
                    op=ALU.add)
                mx = sm3p.tile([128, GSIZE], f32, tag="mx")
                nc.vector.tensor_reduce(
                    mx[:, :ng], h3[:, :W].rearrange("p (t f) -> p t f", f=F3),
                    axis=AX.X, op=ALU.max)
                nc.vector.tensor_tensor(
                    out=h3[:, :W].rearrange("p (t f) -> p t f", f=F3),
                    in0=h3[:, :W].rearrange("p (t f) -> p t f", f=F3),
                    in1=mx[:, :ng].rearrange("p (t o) -> p t o", o=1)
                        .to_broadcast([128, ng, F3]),
                    op=ALU.subtract)
                ex = sm3p.tile([128, GSIZE * F3], f32, tag="ex")
                nc.scalar.activation(ex[:, :W], h3[:, :W], ACT.Exp)
                se = sm3p.tile([128, GSIZE], f32, tag="se")
                nc.vector.tensor_reduce(
                    se[:, :ng], ex[:, :W].rearrange("p (t f) -> p t f", f=F3),
                    axis=AX.X, op=ALU.add)
                ls = sm3p.tile([128, GSIZE], f32, tag="ls")
                nc.scalar.activation(ls[:, :ng], se[:, :ng], ACT.Ln)
                nc.vector.tensor_tensor(
                    out=h3[:, :W].rearrange("p (t f) -> p t f", f=F3),
                    in0=h3[:, :W].rearrange("p (t f) -> p t f", f=F3),
                    in1=ls[:, :ng].rearrange("p (t o) -> p t o", o=1)
                        .to_broadcast([128, ng, F3]),
                    op=ALU.subtract)
                nc.sync.dma_start(
                    t_out.ap()[c0:c0 + gw].rearrange("(t p) f -> p t f", p=128),
                    h3[:, :W].rearrange("p (t f) -> p t f", f=F3))

            agg_layer(zfull3, F3, False, zr3_d, None, "L3agg", final_cb=l3_final)

    nc.compile()
    return nc


_PROG_CACHE = {}


def _in_maps(pp, inputs):
    x = np.asarray(inputs["x"], np.float32)
    iota = np.broadcast_to(np.arange(128, dtype=np.float32)[None, :], (128, 128))
    b3rep = np.broadcast_to(np.asarray(inputs["b3"], np.float32)[None, :], (128, F3)).copy()
    common = {
        "iota": _bf16(iota),
        "W1l": _bf16(inputs["W1l"]),
        "W1r": _bf16(inputs["W1r"]),
        "W2l": _bf16(inputs["W2l"]),
        "W2r": _bf16(inputs["W2r"]),
        "W3l": _bf16(inputs["W3l"]),
        "W3r": _bf16(inputs["W3r"]),
        "g1": np.asarray(inputs["g1"], np.float32)[:, None].copy(),
        "be1": np.asarray(inputs["be1"], np.float32)[:, None].copy(),
        "g2": np.asarray(inputs["g2"], np.float32)[:, None].copy(),
        "be2": np.asarray(inputs["be2"], np.float32)[:, None].copy(),
        "b3rep": b3rep,
    }
    in_maps = []
    for c in range(NCORES):
        xT = np.zeros((FIN, NPAD), np.float32)
        xT[:, :NPC] = x[c * NPC:(c + 1) * NPC].T
        m = dict(common)
        m["xT"] = _bf16(xT)
        m["gidx"] = pp["idx_all"][c]
        m["dstrel"] = _bf16(pp["dstrel_all"][c])
        m["rcnt_nm"] = pp["rcnt_nm"][c]
        m["rcnt_fm"] = np.broadcast_to(pp["rcnt_row"][c][None, :], (64, NPAD)).copy()
        in_maps.append(m)
    return in_maps


def kernel(**inputs):
    edge_index = np.asarray(inputs["edge_index"])
    pp = _preprocess(edge_index)
    key = (pp["npieces"], pp["nch_call"].tobytes())
    if key not in _PROG_CACHE:
        _PROG_CACHE[key] = _build_program(pp)
    nc = _PROG_CACHE[key]
    in_maps = _in_maps(pp, inputs)
    from concourse.bass_utils import run_bass_kernel_spmd
    res = run_bass_kernel_spmd(nc, in_maps, core_ids=list(range(NCORES)))
    return np.concatenate([res.results[c]["out"][:NPC] for c in range(NCORES)], axis=0)


# revision 13
# speedup vs baseline: 3.3163x; 1.0231x over previous
"""GraphSAGE 3-layer GNN forward pass on 8 Trainium2 NeuronCores.

Sharding: nodes split by range across 8 cores (graph/data parallel).
Per layer the message table z = h @ Wl is computed shard-wise (bf16, rows
padded to 128 cols = 256B) and AllGathered into a replicated DRAM table;
each core aggregates the edges whose dst is in its shard: dma_gather pulls
z[src] rows (256B) into SBUF and a one-hot matmul on the tensor engine does
the segment-sum into PSUM (feature-major for layers 1/2, node-major for
layer 3). Mean-normalization (1/deg), the self term h @ Wr (fp32), BatchNorm
(stats AllReduced), ReLU and a batched log_softmax run on vector/scalar.

int16 gather indices only reach 32768 rows, so the table lives in 4 bucket
tensors; bucket b = concat over cores of each shard's b-th quarter (tile-
aligned 3200/3200/3200/2944 rows), so one sub-AllGather per quarter fills a
bucket as soon as the z phase passes that quarter — gathers for bucket 0
start while later quarters are still being computed/gathered.

Edge schedule: per (group of GSIZE dst tiles, src bucket) call, edges are
sorted by dst tile and packed contiguously (chunks may straddle tiles; the
union-of-cores (chunk, tile) piece schedule drives per-piece one-hots whose
dstrel is -1 outside the piece). Trailing idx slots are 0 (gathered then
zeroed by the one-hot); contiguous packing keeps the pad at ~4%.
Gather calls rotate over 4 SWDGE queues so the 4 Q7 core pairs overlap
descriptor generation (the per-edge Q7 descriptor work is the wall).
"""

import numpy as np

# ---------------- problem constants (hardcoded per contract) ----------------
N = 100000
E = 1600000
FIN = 200
NCORES = 8
NPC = N // NCORES            # 12500 nodes per core
NT = 98                      # dst tiles of 128 nodes per core
NPAD = NT * 128              # 12544
SHARD = NPAD                 # table rows contributed per core
NBUCK = 4
QS = [3200, 3200, 3200, 2944]          # shard quarter sizes (tile-aligned)
QSTART = [0, 3200, 6400, 9600]
BROWS = [q * NCORES for q in QS]       # bucket rows (max 25600 < 32768)
F1, F2, F3 = 64, 32, 17
EPS = 1e-5

# ---------------- tunables ----------------
NQ = 4                # SWDGE queues used for gather calls
NSQ = 4               # queues declared
GSIZE = 4             # dst tiles per PSUM accumulation group
PBATCH = 16           # pieces per one-hot build DVE op
GBUFS = 8             # gather buffer pool depth


def _bf16(x):
    import ml_dtypes
    return np.asarray(x).astype(ml_dtypes.bfloat16)


def _wrap16(idx_flat):
    """dma_gather index layout: position i -> partition i%16, col i//16,
    replicated across the 8 q7 core pairs (128 partitions)."""
    n = idx_flat.shape[0]
    w = idx_flat.reshape(n // 16, 16).T.copy()
    return np.tile(w, (8, 1))


def _preprocess(edge_index):
    src = np.asarray(edge_index[0], dtype=np.int64)
    dst = np.asarray(edge_index[1], dtype=np.int64)
    # bucket b of the table = concat over cores of each shard's b-th quarter,
    # so a sub-AllGather of shard quarter b fills bucket b alone.
    src_core = src // NPC
    src_loc = src % NPC
    bucket = np.minimum(src_loc // 3200, 3)
    qs_arr = np.array(QS)[bucket]
    rel = src_core * qs_arr + (src_loc - np.array(QSTART)[bucket])

    dst_core = dst // NPC
    dloc = dst - dst_core * NPC
    tile_e = dloc >> 7
    dstrel_e = dloc & 127

    groups = [list(range(g, min(g + GSIZE, NT))) for g in range(0, NT, GSIZE)]
    ngroups = len(groups)
    gi_of_tile = np.zeros(NT, np.int64)
    for gi, g in enumerate(groups):
        for t in g:
            gi_of_tile[t] = gi
    ncalls = ngroups * NBUCK

    # per-core, per-call edge lists sorted by tile
    per_core = []
    cnt = np.zeros((NCORES, ncalls), np.int64)
    for c in range(NCORES):
        m = dst_core == c
        gi_e = gi_of_tile[tile_e[m]]
        call_e = gi_e * NBUCK + bucket[m]
        order = np.lexsort((tile_e[m], call_e))
        per_core.append({
            "call": call_e[order],
            "tile": tile_e[m][order],
            "rel": rel[m][order],
            "dstrel": dstrel_e[m][order],
            "cnt_node": np.bincount(dloc[m], minlength=NPC),
        })
        cnt[c] = np.bincount(call_e, minlength=ncalls)

    nch_call = np.maximum((cnt.max(axis=0) + 127) // 128, 1)   # [ncalls]

    # union piece schedule: per call, sorted set of (chunk, tile)
    pieces_per_call = []
    for ci in range(ncalls):
        pieces = set()
        for c in range(NCORES):
            ck = per_core[c]
            sel = np.nonzero(ck["call"] == ci)[0]
            if len(sel) == 0:
                continue
            pos = np.arange(len(sel))
            chunks = pos >> 7
            tiles = ck["tile"][sel]
            pieces.update(zip(chunks.tolist(), tiles.tolist()))
        gi = ci // NBUCK
        if not pieces:
            pieces = {(0, groups[gi][0])}
        pieces_per_call.append(sorted(pieces))

    piece_chunk, piece_tile = [], []
    piece_start, piece_stop = [], []
    call_pstart = np.zeros(ncalls + 1, np.int64)
    for gi in range(ngroups):
        for b in range(NBUCK):
            ci = gi * NBUCK + b
            call_pstart[ci] = len(piece_chunk)
            for k, (ch, t) in enumerate(pieces_per_call[ci]):
                piece_chunk.append(ch)
                piece_tile.append(t)
                piece_start.append(b == 0 and k == 0)
                piece_stop.append(b == NBUCK - 1 and k == len(pieces_per_call[ci]) - 1)
    call_pstart[ncalls] = len(piece_chunk)
    npieces = len(piece_chunk)

    idx_cols = int(nch_call.sum()) * 8
    idx_all = np.zeros((NCORES, 128, idx_cols), np.int16)
    dstrel_all = np.full((NCORES, 128, npieces), -1.0, np.float32)
    rcnt_row = np.zeros((NCORES, NPAD), np.float32)
    rcnt_nm = np.zeros((NCORES, 128, NT), np.float32)
    for c in range(NCORES):
        ck = per_core[c]
        idx_parts = []
        for ci in range(ncalls):
            sel = np.nonzero(ck["call"] == ci)[0]
            nidx = int(nch_call[ci]) * 128
            flat = np.zeros(nidx, np.int16)
            flat[:len(sel)] = ck["rel"][sel].astype(np.int16)
            idx_parts.append(_wrap16(flat))
            pos = np.arange(len(sel))
            chunks = pos >> 7
            lanes = pos & 127
            tiles = ck["tile"][sel]
            p0 = call_pstart[ci]
            pmap = {(ch, t): k for k, (ch, t) in enumerate(pieces_per_call[ci])}
            pidx = np.fromiter((pmap[(ch, t)] for ch, t in zip(chunks.tolist(), tiles.tolist())),
                               np.int64, count=len(sel))
            dstrel_all[c][lanes, p0 + pidx] = ck["dstrel"][sel].astype(np.float32)
        idx_all[c] = np.concatenate(idx_parts, axis=1)
        rc_pad = np.ones(NPAD, np.float32)
        rc_pad[:NPC] = 1.0 / np.maximum(ck["cnt_node"], 1).astype(np.float32)
        rcnt_row[c] = rc_pad
        rcnt_nm[c] = rc_pad.reshape(NT, 128).T

    return {
        "groups": groups, "ncalls": ncalls, "nch_call": nch_call,
        "call_pstart": call_pstart, "npieces": npieces,
        "piece_chunk": np.array(piece_chunk), "piece_tile": np.array(piece_tile),
        "piece_start": np.array(piece_start), "piece_stop": np.array(piece_stop),
        "idx_all": idx_all, "idx_cols": idx_cols, "dstrel_all": dstrel_all,
        "rcnt_row": rcnt_row, "rcnt_nm": rcnt_nm,
    }


def _build_program(pp):
    import concourse.bacc as bacc
    import concourse.tile as tile
    import concourse.bass as bass
    import concourse.mybir as mybir

    f32 = mybir.dt.float32
    bf16 = mybir.dt.bfloat16
    AX = mybir.AxisListType
    ALU = mybir.AluOpType
    ACT = mybir.ActivationFunctionType

    groups = pp["groups"]
    ngroups = len(groups)
    ncalls = pp["ncalls"]
    nch_call = pp["nch_call"]
    call_pstart = pp["call_pstart"]
    npieces = pp["npieces"]
    piece_chunk = pp["piece_chunk"]
    piece_tile = pp["piece_tile"]
    piece_start = pp["piece_start"]
    piece_stop = pp["piece_stop"]
    idx_cols = pp["idx_cols"]
    MCC = int(nch_call.max())
    GW = GSIZE * 128

    nc = bacc.Bacc("TRN2", target_bir_lowering=False, debug=False,
                   num_devices=NCORES, num_swdge_queues=NSQ)

    # ---------------- I/O ----------------
    t_xT = nc.dram_tensor("xT", [FIN, NPAD], bf16, kind="ExternalInput")
    t_idx = nc.dram_tensor("gidx", [128, idx_cols], mybir.dt.int16, kind="ExternalInput")
    t_dstrel = nc.dram_tensor("dstrel", [128, npieces], bf16, kind="ExternalInput")
    t_rcnt_nm = nc.dram_tensor("rcnt_nm", [128, NT], f32, kind="ExternalInput")
    t_rcnt_fm = nc.dram_tensor("rcnt_fm", [64, NPAD], f32, kind="ExternalInput")
    t_iota = nc.dram_tensor("iota", [128, 128], bf16, kind="ExternalInput")
    t_W1l = nc.dram_tensor("W1l", [FIN, F1], bf16, kind="ExternalInput")
    t_W1r = nc.dram_tensor("W1r", [FIN, F1], bf16, kind="ExternalInput")
    t_W2l = nc.dram_tensor("W2l", [F1, F2], bf16, kind="ExternalInput")
    t_W2r = nc.dram_tensor("W2r", [F1, F2], bf16, kind="ExternalInput")
    t_W3l = nc.dram_tensor("W3l", [F2, F3], bf16, kind="ExternalInput")
    t_W3r = nc.dram_tensor("W3r", [F2, F3], bf16, kind="ExternalInput")
    t_g1 = nc.dram_tensor("g1", [F1, 1], f32, kind="ExternalInput")
    t_be1 = nc.dram_tensor("be1", [F1, 1], f32, kind="ExternalInput")
    t_g2 = nc.dram_tensor("g2", [F2, 1], f32, kind="ExternalInput")
    t_be2 = nc.dram_tensor("be2", [F2, 1], f32, kind="ExternalInput")
    t_b3 = nc.dram_tensor("b3rep", [128, F3], f32, kind="ExternalInput")
    t_out = nc.dram_tensor("out", [NPAD, F3], f32, kind="ExternalOutput")

    # per-quarter shard and bucket-table tensors (separate tensors so the
    # sub-AllGather b depends only on quarter b's writes)
    shardq = [[nc.dram_tensor(f"shard{li}q{b}", [QS[b], 128], bf16, kind="Internal")
               for b in range(NBUCK)] for li in range(3)]
    zfullq = [[nc.dram_tensor(f"zfull{li}q{b}", [BROWS[b], 128], bf16,
                              kind="Internal", addr_space="Shared")
               for b in range(NBUCK)] for li in range(3)]
    zrT1_d = nc.dram_tensor("zrT1", [64, NPAD], f32, kind="Internal")
    zrT2_d = nc.dram_tensor("zrT2", [F2, NPAD], f32, kind="Internal")
    zr3_d = nc.dram_tensor("zr3", [NPAD, F3], f32, kind="Internal")
    hT1_d = nc.dram_tensor("hT1", [64, NPAD], f32, kind="Internal")
    hT2_d = nc.dram_tensor("hT2", [F2, NPAD], f32, kind="Internal")
    bn_in1 = nc.dram_tensor("bn_in1", [F1, 2], f32, kind="Internal")
    bn_out1 = nc.dram_tensor("bn_out1", [F1, 2], f32, kind="Internal", addr_space="Shared")
    bn_in2 = nc.dram_tensor("bn_in2", [F2, 2], f32, kind="Internal")
    bn_out2 = nc.dram_tensor("bn_out2", [F2, 2], f32, kind="Internal", addr_space="Shared")

    RG = [list(range(NCORES))]

    # tile-aligned (group-row-range, quarter) write segments
    def quarter_segments(c0, gw):
        segs = []
        r = c0
        while r < c0 + gw:
            b = min(r // 3200, 3)
            re = min(c0 + gw, QSTART[b] + QS[b])
            segs.append((b, r, re))
            r = re
        return segs

    with tile.TileContext(nc) as tc:
        with tc.tile_pool(name="const", bufs=1) as constp, \
             tc.tile_pool(name="wpool", bufs=1) as wpool, \
             tc.tile_pool(name="stage", bufs=2) as stagep, \
             tc.tile_pool(name="sm3", bufs=3) as sm3p, \
             tc.tile_pool(name="slab", bufs=2) as slabp, \
             tc.tile_pool(name="gbuf", bufs=GBUFS) as gbufp, \
             tc.tile_pool(name="pbuf", bufs=3) as pbufp, \
             tc.tile_pool(name="zpsum", bufs=2, space="PSUM") as zpsum, \
             tc.tile_pool(name="spsum", bufs=2, space="PSUM") as spsum, \
             tc.tile_pool(name="small", bufs=1) as smallp:

            # ---- constants
            iota = constp.tile([128, 128], bf16)
            nc.sync.dma_start(iota[:], t_iota.ap())
            idx_t = constp.tile([128, idx_cols], mybir.dt.int16)
            nc.sync.dma_start(idx_t[:], t_idx.ap())
            dstrel_t = constp.tile([128, npieces], bf16)
            nc.sync.dma_start(dstrel_t[:], t_dstrel.ap())
            rcnt_nm_t = constp.tile([128, NT], f32)
            nc.sync.dma_start(rcnt_nm_t[:], t_rcnt_nm.ap())
            b3rep = constp.tile([128, F3], f32)
            nc.sync.dma_start(b3rep[:], t_b3.ap())

            def wload(name, tt, shape, dt=bf16):
                w = wpool.tile(shape, dt, tag=name)
                nc.sync.dma_start(w[:], tt)
                return w

            W1l_a = wload("w1la", t_W1l.ap()[:128], [128, F1])
            W1l_b = wload("w1lb", t_W1l.ap()[128:], [72, F1])
            W1r_a = wload("w1ra", t_W1r.ap()[:128], [128, F1])
            W1r_b = wload("w1rb", t_W1r.ap()[128:], [72, F1])
            W2l_t = wload("w2l", t_W2l.ap(), [F1, F2])
            W2r_t = wload("w2r", t_W2r.ap(), [F1, F2])
            W3l_t = wload("w3l", t_W3l.ap(), [F2, F3])
            W3r_t = wload("w3r", t_W3r.ap(), [F2, F3])
            g1_t = wload("g1", t_g1.ap(), [F1, 1], f32)
            be1_t = wload("be1", t_be1.ap(), [F1, 1], f32)
            g2_t = wload("g2", t_g2.ap(), [F2, 1], f32)
            be2_t = wload("be2", t_be2.ap(), [F2, 1], f32)

            # pre-zero rotating pools whose stale regions reach matmuls/DMA
            for _ in range(GBUFS):
                gz = gbufp.tile([128, MCC, 128], bf16, tag="gb")
                nc.vector.memzero(gz[:])
            for _ in range(3):
                zz = sm3p.tile([128, GSIZE, 128], bf16, tag="zslab")
                nc.vector.memzero(zz[:])

            def zwrite(li, g, c0, gw, zsl):
                for b, r, re in quarter_segments(c0, gw):
                    t0 = (r - c0) // 128
                    t1 = (re - c0) // 128
                    nc.sync.dma_start(
                        shardq[li][b].ap()[r - QSTART[b]:re - QSTART[b]]
                            .rearrange("(t p) f -> p t f", p=128),
                        zsl[:, t0:t1, :])

            # ================= layer-1 z phase =================
            with nc.named_scope("L1z"):
                for gi, g in enumerate(groups):
                    gw = len(g) * 128
                    c0 = g[0] * 128
                    xa = slabp.tile([128, GW], bf16, tag="xa")
                    xb = slabp.tile([72, GW], bf16, tag="xb")
                    nc.sync.dma_start(xa[:, :gw], t_xT.ap()[:128, c0:c0 + gw])
                    nc.sync.dma_start(xb[:, :gw], t_xT.ap()[128:, c0:c0 + gw])
                    zr_sl = stagep.tile([64, GW], f32, tag="zrslab")
                    zsl = sm3p.tile([128, GSIZE, 128], bf16, tag="zslab")
                    for ti, t in enumerate(g):
                        xs_a = xa[:, ti * 128:(ti + 1) * 128]
                        xs_b = xb[:, ti * 128:(ti + 1) * 128]
                        pz = zpsum.tile([128, 128], f32, tag="zps")
                        nc.tensor.matmul(pz[:, :F1], xs_a, W1l_a[:], start=True, stop=False)
                        nc.tensor.matmul(pz[:, :F1], xs_b, W1l_b[:], start=False, stop=True)
                        nc.scalar.copy(zsl[:, ti, 0:F1], pz[:, :F1])
                        pr = zpsum.tile([128, 128], f32, tag="zps")
                        nc.tensor.matmul(pr[:F1, :], W1r_a[:], xs_a, start=True, stop=False)
                        nc.tensor.matmul(pr[:F1, :], W1r_b[:], xs_b, start=False, stop=True)
                        nc.scalar.copy(zr_sl[:, ti * 128:(ti + 1) * 128], pr[:F1, :])
                    zwrite(0, g, c0, gw, zsl)
                    nc.sync.dma_start(zrT1_d.ap()[:, c0:c0 + gw], zr_sl[:, :gw])

            def allgather(li, scope):
                with nc.named_scope(scope):
                    for b in range(NBUCK):
                        nc.gpsimd.collective_compute(
                            "AllGather", ALU.bypass, replica_groups=RG,
                            ins=[shardq[li][b].ap()], outs=[zfullq[li][b].ap()])

            allgather(0, "AG1")

            # ========== generic gather/aggregate ==========
            def agg_layer(li, Fw, fm, zr_src, h_sink, scope, final_cb=None):
                stat_parts = []
                with nc.named_scope(scope):
                    cur_ps = None
                    for ci in range(ncalls):
                        gi, b = ci // NBUCK, ci % NBUCK
                        g = groups[gi]
                        gw = len(g) * 128
                        c0 = g[0] * 128
                        nch = int(nch_call[ci])
                        qs8 = int(nch_call[:ci].sum()) * 8
                        if b == 0:
                            if fm:
                                cur_ps = spsum.tile([Fw, GW], f32, tag="sacc")
                            else:
                                cur_ps = spsum.tile([128, GSIZE * F3], f32, tag="sacc3")
                        ps = cur_ps
                        gb = gbufp.tile([128, MCC, 128], bf16, tag="gb")
                        nc.gpsimd.dma_gather(
                            out_ap=gb[:, :nch, :],
                            in_ap=zfullq[li][b].ap(),
                            idxs_ap=idx_t[:, qs8:qs8 + nch * 8],
                            num_idxs=nch * 128, num_idxs_reg=nch * 128,
                            elem_size=128, single_packet=False,
                            queue_num=ci % NQ)
                        p0 = int(call_pstart[ci])
                        pend = int(call_pstart[ci + 1])
                        poff = p0
                        while poff < pend:
                            bs = min(PBATCH, pend - poff)
                            P = pbufp.tile([128, PBATCH * 128], bf16, tag="P")
                            nc.vector.tensor_tensor(
                                out=P[:, :bs * 128].rearrange("p (g v) -> p g v", g=bs),
                                in0=dstrel_t[:, poff:poff + bs]
                                    .to_broadcast([128, bs, 128]),
                                in1=iota[:].rearrange("p (o v) -> p o v", o=1)
                                    .to_broadcast([128, bs, 128]),
                                op=ALU.is_equal)
                            for j in range(bs):
                                pj = poff + j
                                ch = int(piece_chunk[pj])
                                t = int(piece_tile[pj])
                                ti = t - g[0]
                                first = bool(piece_start[pj])
                                last = bool(piece_stop[pj])
                                if fm:
                                    nc.tensor.matmul(
                                        ps[:, ti * 128:(ti + 1) * 128],
                                        gb[:, ch, 0:Fw],
                                        P[:, j * 128:(j + 1) * 128],
                                        start=first, stop=last, skip_group_check=True)
                                else:
                                    nc.tensor.matmul(
                                        ps[:, ti * F3:(ti + 1) * F3],
                                        P[:, j * 128:(j + 1) * 128],
                                        gb[:, ch, 0:F3],
                                        start=first, stop=last, skip_group_check=True)
                            poff += bs
                        if b == NBUCK - 1:
                            if fm:
                                rc_sl = slabp.tile([64, GW], f32, tag="rcsl")
                                nc.sync.dma_start(rc_sl[:Fw, :gw], t_rcnt_fm.ap()[:Fw, c0:c0 + gw])
                                zr_sl2 = slabp.tile([64, GW], f32, tag="zrsl2")
                                nc.sync.dma_start(zr_sl2[:Fw, :gw], zr_src[:, c0:c0 + gw])
                                hsl = stagep.tile([64, GW], f32, tag="hsl")
                                nc.vector.tensor_mul(hsl[:Fw, :gw], ps[:, :gw], rc_sl[:Fw, :gw])
                                nc.vector.tensor_add(hsl[:Fw, :gw], hsl[:Fw, :gw], zr_sl2[:Fw, :gw])
                                s_p = smallp.tile([Fw, 2], f32, tag=f"stat_{scope}_{gi}")
                                nc.vector.tensor_reduce(s_p[:, 0:1], hsl[:Fw, :gw],
                                                        axis=AX.X, op=ALU.add)
                                sq_scr = stagep.tile([64, GW], f32, tag="sqscr")
                                nc.scalar.activation(sq_scr[:Fw, :gw], hsl[:Fw, :gw],
                                                     ACT.Square, accum_out=s_p[:, 1:2])
                                stat_parts.append(s_p)
                                nc.sync.dma_start(h_sink.ap()[:, c0:c0 + gw], hsl[:Fw, :gw])
                            else:
                                final_cb(ps, g, gw, c0)
                return stat_parts

            def bn_finalize(stat_parts, Fw, bn_in, bn_out, g_t, be_t, scope):
                with nc.named_scope(scope):
                    np_ = len(stat_parts)
                    stk = smallp.tile([Fw, 2 * np_], f32, tag=f"stk_{scope}")
                    for i, s_p in enumerate(stat_parts):
                        nc.vector.tensor_copy(stk[:, 2 * i:2 * i + 2], s_p[:])
                    tot = smallp.tile([Fw, 2], f32, tag=f"tot_{scope}")
                    v = stk[:].rearrange("f (i two) -> f two i", two=2)
                    nc.vector.tensor_reduce(tot[:, 0:1], v[:, 0:1, :], axis=AX.X, op=ALU.add)
                    nc.vector.tensor_reduce(tot[:, 1:2], v[:, 1:2, :], axis=AX.X, op=ALU.add)
                    nc.sync.dma_start(bn_in.ap(), tot[:])
                    nc.gpsimd.collective_compute(
                        "AllReduce", ALU.add, replica_groups=RG,
                        ins=[bn_in.ap()], outs=[bn_out.ap()])
                    red = smallp.tile([Fw, 2], f32, tag=f"red_{scope}")
                    nc.sync.dma_start(red[:], bn_out.ap())
                    mean = smallp.tile([Fw, 1], f32, tag=f"mean_{scope}")
                    nc.vector.tensor_scalar_mul(mean[:], red[:, 0:1], 1.0 / N)
                    ex2 = smallp.tile([Fw, 1], f32, tag=f"ex2_{scope}")
                    nc.vector.tensor_scalar_mul(ex2[:], red[:, 1:2], 1.0 / N)
                    var = smallp.tile([Fw, 1], f32, tag=f"var_{scope}")
                    nc.vector.tensor_mul(var[:], mean[:], mean[:])
                    nc.vector.tensor_sub(var[:], ex2[:], var[:])
                    nc.vector.tensor_scalar_add(var[:], var[:], EPS)
                    std = smallp.tile([Fw, 1], f32, tag=f"std_{scope}")
                    nc.scalar.sqrt(std[:], var[:])
                    rstd = smallp.tile([Fw, 1], f32, tag=f"rstd_{scope}")
                    nc.vector.reciprocal(rstd[:], std[:])
                    scal = smallp.tile([Fw, 1], f32, tag=f"scal_{scope}")
                    nc.vector.tensor_mul(scal[:], g_t[:], rstd[:])
                    shift = smallp.tile([Fw, 1], f32, tag=f"shift_{scope}")
                    nc.vector.tensor_mul(shift[:], mean[:], scal[:])
                    nc.vector.tensor_sub(shift[:], be_t[:], shift[:])
                    return scal, shift

            stats1 = agg_layer(0, F1, True, zrT1_d.ap(), hT1_d, "L1agg")
            scal1, shift1 = bn_finalize(stats1, F1, bn_in1, bn_out1, g1_t, be1_t, "BN1")

            # ================= layer-2 z phase =================
            with nc.named_scope("L2z"):
                for gi, g in enumerate(groups):
                    gw = len(g) * 128
                    c0 = g[0] * 128
                    hs = slabp.tile([64, GW], f32, tag="hs")
                    nc.sync.dma_start(hs[:F1, :gw], hT1_d.ap()[:, c0:c0 + gw])
                    hsb = slabp.tile([64, GW], bf16, tag="hsb")
                    nc.scalar.activation(hsb[:F1, :gw], hs[:F1, :gw], ACT.Relu,
                                         bias=shift1[:], scale=scal1[:])
                    if g[-1] == NT - 1:
                        nc.vector.memzero(hsb[:F1, NPC - c0:gw])
                    zr_sl = stagep.tile([64, GW], f32, tag="zrslab")
                    zsl = sm3p.tile([128, GSIZE, 128], bf16, tag="zslab")
                    for ti, t in enumerate(g):
                        hst = hsb[:F1, ti * 128:(ti + 1) * 128]
                        pz = zpsum.tile([128, 128], f32, tag="zps")
                        nc.tensor.matmul(pz[:, :F2], hst, W2l_t[:], start=True, stop=True)
                        nc.scalar.copy(zsl[:, ti, 0:F2], pz[:, :F2])
                        pr = zpsum.tile([128, 128], f32, tag="zps")
                        nc.tensor.matmul(pr[:F2, :], W2r_t[:], hst, start=True, stop=True)
                        nc.scalar.copy(zr_sl[:F2, ti * 128:(ti + 1) * 128], pr[:F2, :])
                    zwrite(1, g, c0, gw, zsl)
                    nc.sync.dma_start(zrT2_d.ap()[:, c0:c0 + gw], zr_sl[:F2, :gw])

            allgather(1, "AG2")

            stats2 = agg_layer(1, F2, True, zrT2_d.ap(), hT2_d, "L2agg")
            scal2, shift2 = bn_finalize(stats2, F2, bn_in2, bn_out2, g2_t, be2_t, "BN2")

            # ================= layer-3 z phase =================
            with nc.named_scope("L3z"):
                for gi, g in enumerate(groups):
                    gw = len(g) * 128
                    c0 = g[0] * 128
                    hs = slabp.tile([64, GW], f32, tag="hs")
                    nc.sync.dma_start(hs[:F2, :gw], hT2_d.ap()[:, c0:c0 + gw])
                    hsb = slabp.tile([64, GW], bf16, tag="hsb")
                    nc.scalar.activation(hsb[:F2, :gw], hs[:F2, :gw], ACT.Relu,
                                         bias=shift2[:], scale=scal2[:])
                    if g[-1] == NT - 1:
                        nc.vector.memzero(hsb[:F2, NPC - c0:gw])
                    zsl = sm3p.tile([128, GSIZE, 128], bf16, tag="zslab")
                    zr3sl = sm3p.tile([128, GSIZE, F3], f32, tag="zr3slab")
                    for ti, t in enumerate(g):
                        hst = hsb[:F2, ti * 128:(ti + 1) * 128]
                        pz = zpsum.tile([128, 128], f32, tag="zps")
                        nc.tensor.matmul(pz[:, :F3], hst, W3l_t[:], start=True, stop=True)
                        nc.scalar.copy(zsl[:, ti, 0:F3], pz[:, :F3])
                        pr = zpsum.tile([128, 128], f32, tag="zps")
                        nc.tensor.matmul(pr[:, :F3], hst, W3r_t[:], start=True, stop=True)
                        nc.scalar.copy(zr3sl[:, ti, :], pr[:, :F3])
                    zwrite(2, g, c0, gw, zsl)
                    nc.sync.dma_start(
                        zr3_d.ap()[c0:c0 + gw].rearrange("(t p) f -> p t f", p=128),
                        zr3sl[:, :len(g), :])

            allgather(2, "AG3")

            def l3_final(ps, g, gw, c0):
                ng = len(g)
                W = ng * F3
                zr_sl3 = slabp.tile([128, GSIZE * F3], f32, tag="zrsl3")
                nc.sync.dma_start(
                    zr_sl3[:, :W].rearrange("p (t f) -> p t f", f=F3),
                    zr3_d.ap()[c0:c0 + gw].rearrange("(t p) f -> p t f", p=128))
                h3 = sm3p.tile([128, GSIZE * F3], f32, tag="h3")
                nc.vector.tensor_tensor(
                    out=h3[:, :W].rearrange("p (t f) -> p t f", f=F3),
                    in0=ps[:, :W].rearrange("p (t f) -> p t f", f=F3),
                    in1=rcnt_nm_t[:, g[0]:g[0] + ng]
                        .rearrange("p (t o) -> p t o", o=1)
                        .to_broadcast([128, ng, F3]),
                    op=ALU.mult)
                nc.vector.tensor_add(h3[:, :W], h3[:, :W], zr_sl3[:, :W])
                nc.vector.tensor_tensor(
                    out=h3[:, :W].rearrange("p (t f) -> p t f", f=F3),
                    in0=h3[:, :W].rearrange("p (t f) -> p t f", f=F3),
                    in1=b3rep[:, :F3].rearrange("p (o f) -> p o f", o=1)
                        .to_broadcast([128, ng, F3]),
                    op=ALU.add)
                mx = sm3p.tile([128, GSIZE], f32, tag="mx")
                nc.vector.tensor_reduce(
                    mx[:, :ng], h3[:, :W].rearrange("p (t f) -> p t f", f=F3),
                    axis=AX.X, op=ALU.max)
                nc.vector.tensor_tensor(
                    out=h3[:, :W].rearrange("p (t f) -> p t f", f=F3),
                    in0=h3[:, :W].rearrange("p (t f) -> p t f", f=F3),
                    in1=mx[:, :ng].rearrange("p (t o) -> p t o", o=1)
                        .to_broadcast([128, ng, F3]),
                    op=ALU.subtract)
                ex = sm3p.tile([128, GSIZE * F3], f32, tag="ex")
                nc.scalar.activation(ex[:, :W], h3[:, :W], ACT.Exp)
                se = sm3p.tile([128, GSIZE], f32, tag="se")
                nc.vector.tensor_reduce(
                    se[:, :ng], ex[:, :W].rearrange("p (t f) -> p t f", f=F3),
                    axis=AX.X, op=ALU.add)
                ls = sm3p.tile([128, GSIZE], f32, tag="ls")
                nc.scalar.activation(ls[:, :ng], se[:, :ng], ACT.Ln)
                nc.vector.tensor_tensor(
                    out=h3[:, :W].rearrange("p (t f) -> p t f", f=F3),
                    in0=h3[:, :W].rearrange("p (t f) -> p t f", f=F3),
                    in1=ls[:, :ng].rearrange("p (t o) -> p t o", o=1)
                        .to_broadcast([128, ng, F3]),
                    op=ALU.subtract)
                nc.sync.dma_start(
                    t_out.ap()[c0:c0 + gw].rearrange("(t p) f -> p t f", p=128),
                    h3[:, :W].rearrange("p (t f) -> p t f", f=F3))

            agg_layer(2, F3, False, zr3_d, None, "L3agg", final_cb=l3_final)

    nc.compile()
    return nc


_PROG_CACHE = {}


def _in_maps(pp, inputs):
    x = np.asarray(inputs["x"], np.float32)
    iota = np.broadcast_to(np.arange(128, dtype=np.float32)[None, :], (128, 128))
    b3rep = np.broadcast_to(np.asarray(inputs["b3"], np.float32)[None, :], (128, F3)).copy()
    common = {
        "iota": _bf16(iota),
        "W1l": _bf16(inputs["W1l"]),
        "W1r": _bf16(inputs["W1r"]),
        "W2l": _bf16(inputs["W2l"]),
        "W2r": _bf16(inputs["W2r"]),
        "W3l": _bf16(inputs["W3l"]),
        "W3r": _bf16(inputs["W3r"]),
        "g1": np.asarray(inputs["g1"], np.float32)[:, None].copy(),
        "be1": np.asarray(inputs["be1"], np.float32)[:, None].copy(),
        "g2": np.asarray(inputs["g2"], np.float32)[:, None].copy(),
        "be2": np.asarray(inputs["be2"], np.float32)[:, None].copy(),
        "b3rep": b3rep,
    }
    in_maps = []
    for c in range(NCORES):
        xT = np.zeros((FIN, NPAD), np.float32)
        xT[:, :NPC] = x[c * NPC:(c + 1) * NPC].T
        m = dict(common)
        m["xT"] = _bf16(xT)
        m["gidx"] = pp["idx_all"][c]
        m["dstrel"] = _bf16(pp["dstrel_all"][c])
        m["rcnt_nm"] = pp["rcnt_nm"][c]
        m["rcnt_fm"] = np.broadcast_to(pp["rcnt_row"][c][None, :], (64, NPAD)).copy()
        in_maps.append(m)
    return in_maps


def kernel(**inputs):
    edge_index = np.asarray(inputs["edge_index"])
    pp = _preprocess(edge_index)
    key = (pp["npieces"], pp["nch_call"].tobytes())
    if key not in _PROG_CACHE:
        _PROG_CACHE[key] = _build_program(pp)
    nc = _PROG_CACHE[key]
    in_maps = _in_maps(pp, inputs)
    from concourse.bass_utils import run_bass_kernel_spmd
    res = run_bass_kernel_spmd(nc, in_maps, core_ids=list(range(NCORES)))
    return np.concatenate([res.results[c]["out"][:NPC] for c in range(NCORES)], axis=0)
